# revision 2
# baseline (speedup 1.0000x reference)
"""Sparse (half-causal) multi-head attention on 8 Trainium2 NeuronCores, v3.

Problem: x[2,2048,1024] -> QKV proj (16 heads, dk=dv=64) -> scores with
half-causal mask (rows <1024 attend cols <1024 dense; rows >=1024 causal)
-> softmax -> out proj.

Sharding: 8 cores = 2 batches x 4 head-groups (4 heads each).  Each core
computes its batch's full QKV for its 4 heads (column-sharded W), attention
for those heads, and a partial output projection (row-sharded Wo).  Host
sums the 4 partials per batch.

v3 changes vs v2 (152.3us):
 - scores in fp8 e4m3 via DoubleRow perf mode with a stride-0 k-tile slot:
   both DR slots read the same data, computing 2x(k.q) at 0.5 cyc/row; the
   doubling folds into the exp scale (0.0625).  Halves scores PE time.
 - PV flipped: out[q,v] = probs[k,q].T @ v[k,v] per 128-q chunk, cost 65
   rows/chunk instead of 128 (the 65-wide v+denominator free dim is the
   cheap side).  Denominator becomes a per-partition scalar: recip +
   tensor_scalar multiply, killing the dscr broadcast-DMA machinery.
 - att transposed back via PE transpose (identity moving operand) with the
   bf16 output bitcast into the just-consumed pv psum bank (no extra bank).
 - no y2 partial: the causal staircase finishes head (1,1) q-chunk j at
   kc 8+j, so the full out-projection (both head pairs) rides the diagonal.
 - most y chunks DMA directly from PSUM with an f32->bf16 casting DMA on
   the gpsimd queue (no DVE copy); the last chunks copy+DMA for a fast tail.
 - work-queue scheduler: QKV chunks 2-3, PV units and out-projections pop
   between scores/exp emissions to keep PE busy while ACT (exp) streams.
"""

import copy as _copy
import heapq
import sys
from collections import deque

if "/opt/trn_rl_repo" not in sys.path:
    sys.path.insert(0, "/opt/trn_rl_repo")

import ml_dtypes
import numpy as np

import concourse.bass as bass  # noqa: F401 (import registers engines)
import concourse.mybir as mybir
import concourse.tile as tile
from concourse import bacc
from concourse.bass_utils import run_bass_kernel_spmd

f32 = mybir.dt.float32
bf16 = mybir.dt.bfloat16
fp8 = mybir.dt.float8e4
AF = mybir.ActivationFunctionType
OP = mybir.AluOpType
PM = mybir.MatmulPerfMode

D = 1024  # d_model
N = 2048  # n_ctx
HG = 256  # head-group width per core (4 heads x 64)

HEADS = [(0, 0), (0, 1), (1, 0), (1, 1)]  # (hp, par)


def stride0(ap):
    """Turn the first singleton non-partition dim into a stride-0 2-count
    dim (the DoubleRow k-tile slot reading the same data twice)."""
    ap2 = _copy.copy(ap)
    lst = ap2.ap
    for i in range(1, len(lst)):
        if lst[i][1] == 1:
            lst[i] = [0, 2]
            return ap2
    raise RuntimeError(f"no singleton dim in {lst}")


def build_nc():
    nc = bacc.Bacc("TRN2", target_bir_lowering=False, debug=False)

    # x and the QKV weights ship as fp8 hi/lo pairs (weights pre-scaled x16
    # on the host; the 1/16 folds into the psum->sbuf copy)
    xth = nc.declare_dram_parameter("xth", [D, N], fp8, isOutput=False)
    xtl = nc.declare_dram_parameter("xtl", [D, N], fp8, isOutput=False)
    wq8 = nc.declare_dram_parameter("wq8", [D, 2, HG], fp8, isOutput=False)
    wk8 = nc.declare_dram_parameter("wk8", [D, 2, HG], fp8, isOutput=False)
    wv8 = nc.declare_dram_parameter("wv8", [D, 2, HG], fp8, isOutput=False)
    bqd = nc.declare_dram_parameter("bq", [HG], f32, isOutput=False)
    bvd = nc.declare_dram_parameter("bv", [HG], bf16, isOutput=False)
    wo = nc.declare_dram_parameter("wo", [HG, D], bf16, isOutput=False)
    trid = nc.declare_dram_parameter("tri", [128, 128], bf16, isOutput=False)
    idmd = nc.declare_dram_parameter("idm", [128, 128], bf16, isOutput=False)
    onesd = nc.declare_dram_parameter("ones", [512], bf16, isOutput=False)
    y = nc.declare_dram_parameter("y", [N, D], bf16, isOutput=True)

    xth_r = xth[:].rearrange("(c p) n -> p c n", p=128)
    xtl_r = xtl[:].rearrange("(c p) n -> p c n", p=128)
    wq_r = wq8[:].rearrange("(c p) hl m -> p c hl m", p=128)
    wk_r = wk8[:].rearrange("(c p) hl m -> p c hl m", p=128)
    wv_r = wv8[:].rearrange("(c p) hl m -> p c hl m", p=128)
    wo_r = wo[:].rearrange("(c p) n -> p c n", p=128)

    with tile.TileContext(nc) as tc:
        with (
            tc.tile_pool(name="persist", bufs=1) as P1,
            tc.tile_pool(name="xtp", bufs=6) as XTP,
            tc.tile_pool(name="ppool", bufs=34) as PP,
            tc.tile_pool(name="aqp", bufs=3) as AQP,
            tc.tile_pool(name="spp", bufs=2) as SPP,
            tc.tile_pool(name="rp", bufs=3) as RP,
            tc.tile_pool(name="yp", bufs=3) as YP,
            tc.tile_pool(name="ps_s", bufs=2, space="PSUM") as PSS,
            tc.tile_pool(name="ps_pv", bufs=2, space="PSUM") as PSPV,
            tc.tile_pool(name="ps_b", bufs=2, space="PSUM") as PSB,
        ):
            # ---------- persistent tiles ----------
            # [part, c-chunk, hi/lo, cols]: the (c, c+1) pairing for DR slots
            # lives in the c dim; hi/lo selects the split
            wq_sb = P1.tile([128, 8, 2, HG], fp8, tag="wq")
            wk_sb = P1.tile([128, 8, 2, HG], fp8, tag="wk")
            wv_sb = P1.tile([128, 8, 2, HG], fp8, tag="wv")
            wo_sb = P1.tile([128, 2, D], bf16, tag="wo")
            bq_sb = P1.tile([128, 2], f32, tag="bq")
            bv_sb = P1.tile([1, HG], bf16, tag="bv")
            ones_sb = P1.tile([1, 512], bf16, tag="ones")
            tri_sb = P1.tile([128, 128], bf16, tag="tri")
            idm_sb = P1.tile([128, 128], bf16, tag="idm")

            ebase = P1.tile([128, 1024], f32, tag="ebase")
            qT8 = P1.tile([128, 2, N], fp8, tag="qT8")
            kT8 = P1.tile([128, 2, N], fp8, tag="kT8")
            v1 = P1.tile([128, 16, 4, 65], bf16, tag="v1")
            att = P1.tile([128, 2, N], bf16, tag="att")

            # ---------- QKV emitters ----------
            xt_tiles = {}

            def emit_load(n4, split=False):
                ns = slice(512 * n4, 512 * n4 + 512)
                a = XTP.tile([128, 8, 512], fp8, tag="xt", name=f"xth{n4}")
                b_ = XTP.tile([128, 8, 512], fp8, tag="xt", name=f"xtl{n4}")
                for h in range(2):
                    cs = slice(4 * h, 4 * h + 4)
                    nc.sync.dma_start(a[:, cs, :], xth_r[:, cs, ns])
                eng = nc.gpsimd if split else nc.sync
                for h in range(2):
                    cs = slice(4 * h, 4 * h + 4)
                    eng.dma_start(b_[:, cs, :], xtl_r[:, cs, ns])
                xt_tiles[n4] = (a, b_)

            def emit_qk_part(which, n4, m, toff, twid, act_copy=False):
                """Q/K projection for a token sub-range via 3-term hi/lo fp8
                DoubleRow (xh@Wh + xl@Wh + xh@Wl); weights are x16 so the
                psum->sbuf copy scales by 1/16 (and adds bq for Q)."""
                ns = slice(512 * n4 + toff, 512 * n4 + toff + twid)
                xh, xl = xt_tiles[n4]
                msl = slice(128 * m, 128 * m + 128)
                w_sb = wq_sb if which == "q" else wk_sb
                ps = PSB.tile(
                    [128, 512], f32, tag="b", name=f"{which}ps{n4}{m}{toff}"
                )
                terms = [(xh, 0), (xl, 0), (xh, 1)]
                for ti, (xs, hl) in enumerate(terms):
                    for j in range(4):
                        nc.tensor.matmul(
                            ps[:, 0:twid],
                            w_sb[:, 2 * j : 2 * j + 2, hl, msl],
                            xs[:, 2 * j : 2 * j + 2, toff : toff + twid],
                            start=(ti == 0 and j == 0),
                            stop=(ti == 2 and j == 3),
                            perf_mode=PM.DoubleRow,
                        )
                if which == "q":
                    with nc.allow_low_precision(reason="fp8 qT"):
                        if act_copy:
                            nc.scalar.activation(
                                qT8[:, m, ns], ps[:, 0:twid], AF.Identity,
                                bias=bq_sb[:, m : m + 1], scale=1.0 / 16.0,
                            )
                        else:
                            nc.vector.tensor_scalar(
                                out=qT8[:, m, ns], in0=ps[:, 0:twid],
                                scalar1=1.0 / 16.0,
                                scalar2=bq_sb[:, m : m + 1],
                                op0=OP.mult, op1=OP.add,
                            )
                else:
                    with nc.allow_low_precision(reason="fp8 kT"):
                        if act_copy:
                            nc.scalar.mul(kT8[:, m, ns], ps[:, 0:twid], 1.0 / 16.0)
                        else:
                            nc.vector.tensor_scalar(
                                out=kT8[:, m, ns], in0=ps[:, 0:twid],
                                scalar1=1.0 / 16.0, scalar2=None, op0=OP.mult,
                            )

            def emit_q(n4, m, act_copy=False):
                emit_qk_part("q", n4, m, 0, 512, act_copy=act_copy)

            def emit_k(n4, m, act_copy=False):
                emit_qk_part("k", n4, m, 0, 512, act_copy=act_copy)

            def emit_v(s, act_copy=False):
                n4 = s // 4
                xh, xl = xt_tiles[n4]
                so = 128 * (s - 4 * n4)
                ps = PSB.tile([128, 256], f32, tag="b", name=f"vps{s}")
                terms = [(xh, 0), (xl, 0), (xh, 1)]
                for ti, (xs, hl) in enumerate(terms):
                    for j in range(4):
                        nc.tensor.matmul(
                            ps[:],
                            xs[:, 2 * j : 2 * j + 2, so : so + 128],
                            wv_sb[:, 2 * j : 2 * j + 2, hl, :],
                            start=(ti == 0 and j == 0),
                            stop=False,
                            perf_mode=PM.DoubleRow,
                        )
                # bias (x16 on host, like the weights)
                nc.tensor.matmul(
                    ps[:], ones_sb[:, :128], bv_sb[:], start=False, stop=True
                )
                with nc.allow_low_precision(reason="bf16 v1"):
                    src = ps[:].rearrange("p (h d) -> p h d", h=4)
                    if act_copy:
                        nc.scalar.mul(v1[:, s, :, 0:64], src, 1.0 / 16.0)
                    else:
                        nc.vector.tensor_scalar(
                            out=v1[:, s, :, 0:64], in0=src,
                            scalar1=1.0 / 16.0, scalar2=None, op0=OP.mult,
                        )

            # ---------- attention ----------
            probs = {}  # (bi, kc) -> p_t

            def emit_scores(bi, hp, par, half, kc):
                q0 = 1024 * half
                base = 64 * par
                o = 0 if (half == 0 or kc < 8) else 128 * (kc - 8)
                # correctness guards: prerequisite projections must be emitted
                for n4 in (2 * half, 2 * half + 1):
                    for toff in (0, 256):
                        ensure(("q", n4, hp, toff))
                ensure(("k", kc // 4, hp, 256 * ((kc % 4) // 2)))
                s_t = PSS.tile([128, 1024], f32, tag="s", name=f"s{bi}_{kc}")
                lhsT = stride0(kT8[base : base + 64, hp : hp + 1,
                                   128 * kc : 128 * kc + 128])
                # matmul out must stay within one PSUM bank (<=512 f32)
                mm = [(o, 512), (512, 1024)] if o < 512 else [(o, 1024)]
                for lo, hi in mm:
                    rhs = stride0(qT8[base : base + 64, hp : hp + 1,
                                      q0 + lo : q0 + hi])
                    nc.tensor.matmul(
                        s_t[:, lo:hi], lhsT, rhs, start=True, stop=True,
                        perf_mode=PM.DoubleRow,
                    )
                p_t = PP.tile([128, 1024], bf16, tag="p", name=f"p{bi}_{kc}")
                w_ = 1024 - o
                if half == 1 and kc <= -1:
                    # offload early dense exps (their probs are consumed ~8
                    # periods later, hiding the DVE->Pool chain latency):
                    # DVE stages scores to SBUF, Pool computes base^s
                    s_sb = SPP.tile(
                        [128, 1024], f32, tag="ss", name=f"ss{bi}_{kc}"
                    )
                    nc.vector.tensor_copy(out=s_sb[:, 0:w_], in_=s_t[:, o:1024])
                    with nc.allow_low_precision(reason="bf16 probs"):
                        nc.gpsimd.tensor_tensor(
                            p_t[:, o:1024], ebase[:, 0:w_], s_sb[:, 0:w_],
                            OP.pow,
                        )
                else:
                    with nc.allow_low_precision(reason="bf16 probs"):
                        # DR stride-0 doubles the score; 1/16 = 0.5 * 1/8
                        nc.scalar.activation(
                            p_t[:, o:1024], s_t[:, o:1024], AF.Exp, scale=0.0625
                        )
                if half == 1 and kc >= 8:
                    # causal mask inside the diagonal 128-block (SBUF-only op
                    # -> Pool engine, which cannot touch PSUM anyway)
                    nc.gpsimd.tensor_tensor(
                        p_t[:, o : o + 128], p_t[:, o : o + 128], tri_sb[:],
                        OP.mult,
                    )
                probs[(bi, kc)] = p_t

            # per-block deferred transpose state: (pv, attq, qc)
            pend_fin = {}

            def emit_pv(bi, hp, par, half, qc, nkc):
                for kc in range(nkc):
                    ensure(("v", kc))
                h2 = 2 * hp + par
                pv = PSPV.tile([128, 512], f32, tag="pv", name=f"pv{bi}_{qc}")
                col = slice(128 * qc, 128 * qc + 128)
                for kc in range(nkc):
                    nc.tensor.matmul(
                        pv[:, 0:65],
                        probs[(bi, kc)][:, col],
                        v1[:, kc, h2, :],
                        start=(kc == 0),
                        stop=(kc == nkc - 1),
                    )
                return pv

            def emit_norm(bi, qc, pv, act=False):
                r = RP.tile([128, 1], f32, tag="r", name=f"r{bi}_{qc}")
                nc.vector.reciprocal(r[:], pv[:, 64:65])
                attq = AQP.tile([128, 64], bf16, tag="aq", name=f"aq{bi}_{qc}")
                with nc.allow_low_precision(reason="bf16 attq"):
                    if act:
                        # ACT is idle in the tail: out = in * r (per-partition)
                        nc.scalar.activation(
                            attq[:], pv[:, 0:64], AF.Copy, scale=r[:]
                        )
                    else:
                        nc.vector.tensor_scalar(
                            out=attq[:], in0=pv[:, 0:64], scalar1=r[:],
                            scalar2=None, op0=OP.mult,
                        )
                return attq

            def emit_finish(hp, par, half, qc, pv, attq, act=False):
                # transpose attq -> [64,128] into the dead pv bank, then copy
                base = 64 * par
                q0 = 1024 * half
                trout = pv[base : base + 64, 128:192].bitcast(bf16)
                nc.tensor.matmul(
                    trout, attq[:], idm_sb[:], start=True, stop=True,
                    is_transpose=True,
                )
                with nc.allow_low_precision(reason="bf16 att"):
                    dst = att[base : base + 64, hp,
                              q0 + 128 * qc : q0 + 128 * qc + 128]
                    if act:
                        nc.scalar.copy(out=dst, in_=trout)
                    else:
                        nc.vector.tensor_copy(out=dst, in_=trout)

            # ---------- output projection ----------
            yt_tiles = {}

            def emit_outproj_seg(s, nseg, late=False):
                if nseg == 0:
                    yt_tiles[s] = YP.tile([128, D], bf16, tag="y", name=f"yt{s}")
                yt = yt_tiles[s]
                ps = PSB.tile([128, 512], f32, tag="b", name=f"yps{s}_{nseg}")
                for hp in range(2):
                    nc.tensor.matmul(
                        ps[:],
                        att[:, hp, 128 * s : 128 * s + 128],
                        wo_sb[:, hp, 512 * nseg : 512 * nseg + 512],
                        start=(hp == 0),
                        stop=(hp == 1),
                    )
                sl = slice(512 * nseg, 512 * nseg + 512)
                with nc.allow_low_precision(reason="bf16 y"):
                    if s >= 13:
                        nc.scalar.copy(out=yt[:, sl], in_=ps[:])
                    else:
                        nc.vector.tensor_copy(out=yt[:, sl], in_=ps[:])
                if nseg == 1:
                    del yt_tiles[s]
                    # dual-queue issue so the sequencer cost doesn't serialize
                    # the drain; keep the slower SWDGE path off the last chunks
                    eng = nc.gpsimd if not late else nc.sync
                    eng.dma_start(y[128 * s : 128 * s + 128, :], yt[:])

            # ============================================================
            # work queue
            # ============================================================
            work = deque()  # (rows, fn, key) — filler units
            done = set()  # keys of emitted filler units
            pv_q = []  # heap of (release_period, seq, rows, fn)
            pv_seq = [0]
            period = [0]
            acct = [0.0]  # carry-over PE-row budget

            def run_unit(rows, fn, key):
                if key is not None:
                    if key in done:
                        return 0
                    done.add(key)
                fn()
                return rows

            def ensure(key):
                """Emit a queued filler unit NOW if it hasn't run yet —
                correctness guard so pacing can never reorder a consumer
                ahead of its producer."""
                if key in done:
                    return
                for i, (rows, fn, k) in enumerate(work):
                    if k == key:
                        del work[i]
                        run_unit(rows, fn, key)
                        acct[0] -= rows
                        return
                raise RuntimeError(f"missing unit {key}")

            def pv_push(release, rows_fn):
                rows, fn = rows_fn
                heapq.heappush(pv_q, (release, pv_seq[0], rows, fn))
                pv_seq[0] += 1

            def pop_rows(alloc):
                # released PV/op units first (their release period guarantees
                # deps are done, so they never stall the in-order PE stream),
                # then ready filler while the carry-over account affords it
                acct[0] = min(acct[0] + alloc, 1.5 * alloc if alloc > 0 else acct[0])
                while pv_q and pv_q[0][0] <= period[0]:
                    _, _, rows, fn = heapq.heappop(pv_q)
                    fn()
                    acct[0] -= rows
                while work and work[0][0] <= acct[0]:
                    rows, fn, key = work.popleft()
                    acct[0] -= run_unit(rows, fn, key)

            def pv_unit(bi, hp, par, half, qc, nkc):
                act = bi >= 7

                def fn():
                    pv = emit_pv(bi, hp, par, half, qc, nkc)
                    attq = emit_norm(bi, qc, pv, act=act)
                    if bi in pend_fin:
                        ppv, pattq, pqc = pend_fin.pop(bi)
                        emit_finish(hp, par, half, pqc, ppv, pattq, act=act)
                        after_finish(bi, half, pqc)
                    pend_fin[bi] = (pv, attq, qc)

                return (65 * nkc + 192, fn)

            def flush_unit(bi, hp, par, half):
                act = bi >= 7

                def fn():
                    if bi in pend_fin:
                        ppv, pattq, pqc = pend_fin.pop(bi)
                        emit_finish(hp, par, half, pqc, ppv, pattq, act=act)
                        after_finish(bi, half, pqc)

                return (192, fn)

            def after_finish(bi, half, qc):
                # out-projections unlock when the LAST head (block bi=3 for
                # half 0, bi=7 for half 1) lands its att chunk
                if bi == 3 and qc == 7:
                    # ration the deadline-free half-0 out-projections across
                    # blocks 4-6 so the diag stretches keep PE fed
                    for i in range(16):
                        s, nseg = i // 2, i % 2
                        pv_push(
                            period[0] + 2 + 4 * i,
                            (1024, lambda s=s, n=nseg: emit_outproj_seg(s, n)),
                        )
                elif bi == 7:
                    s = 8 + qc
                    for nseg in range(2):
                        pv_push(
                            period[0],
                            (1024, lambda s=s, n=nseg:
                             emit_outproj_seg(s, n, late=(s >= 12))),
                        )

            # ============================================================
            # emission schedule
            # ============================================================
            # --- ramp: stream x/weights, QKV chunks 0-1 ---
            nc.gpsimd.memzero(ones_sb[:])
            with nc.allow_low_precision(reason="bf16 ones"):
                nc.gpsimd.tensor_scalar_add(ones_sb[:], ones_sb[:], 1.0)
            nc.gpsimd.memset(ebase[:], float(np.exp(0.0625)))
            nc.scalar.dma_start(wq_sb[:, 0:4, :, :], wq_r[:, 0:4, :, :])
            emit_load(0, split=True)
            nc.scalar.dma_start(wq_sb[:, 4:8, :, :], wq_r[:, 4:8, :, :])
            nc.scalar.dma_start(wk_sb[:, 0:4, :, :], wk_r[:, 0:4, :, :])
            nc.scalar.dma_start(wk_sb[:, 4:8, :, :], wk_r[:, 4:8, :, :])
            nc.scalar.dma_start(wv_sb[:], wv_r[:])
            nc.gpsimd.dma_start(bq_sb[:], bqd[:].rearrange("(m p) -> p m", p=128))
            nc.gpsimd.dma_start(bv_sb[:], bvd[None, :])
            nc.scalar.dma_start(tri_sb[:], trid[:])
            nc.scalar.dma_start(idm_sb[:], idmd[:])
            # warmup matmuls: climb the PE p-state while DMAs stream
            for w in range(8):
                junk = PSB.tile([128, 512], f32, tag="b", name=f"warm{w}")
                nc.tensor.matmul(
                    junk[:], ones_sb[0:1, :128], ones_sb[:], start=True, stop=True
                )
            # minimal pre-B0 projections: only hp0 q (both token halves of
            # q-half0) and the first K chunk; everything else becomes filler
            emit_q(0, 0, act_copy=True)
            emit_k(0, 0, act_copy=True)
            emit_load(1)
            emit_q(1, 0, act_copy=True)
            nc.sync.dma_start(v1[:, :, :, 64:65], onesd[0:64].partition_broadcast(128))
            emit_load(2)
            emit_load(3)
            nc.scalar.dma_start(wo_sb[:], wo_r[:])

            # ramp already emitted these:
            done.update({("q", 0, 0, 0), ("q", 0, 0, 256),
                         ("k", 0, 0, 0), ("k", 0, 0, 256),
                         ("q", 1, 0, 0), ("q", 1, 0, 256)})

            def push_qk(which, n4, m):
                for toff in (0, 256):
                    work.append(
                        (1536,
                         lambda n4=n4, m=m, toff=toff:
                         emit_qk_part(which, n4, m, toff, 256),
                         (which, n4, m, toff))
                    )

            # --- remaining QKV queued as filler, in deadline order ---
            # v0-7 + k(1,0): B0; m=1 units: B2 kc0; q n4 2-3: B4; v8-15: B5
            def push_v(s):
                work.append((1792, lambda s=s: emit_v(s), ("v", s)))

            push_v(0)
            push_v(1)
            push_qk("k", 1, 0)
            for s in range(2, 8):
                push_v(s)
            push_qk("q", 0, 1)
            push_qk("k", 0, 1)
            push_qk("q", 1, 1)
            push_qk("k", 1, 1)
            push_qk("q", 2, 0)
            push_qk("q", 3, 0)
            push_qk("q", 2, 1)
            push_qk("q", 3, 1)
            push_qk("k", 2, 0)
            push_qk("k", 3, 0)
            for s in range(8, 12):
                push_v(s)
            push_qk("k", 2, 1)
            push_qk("k", 3, 1)
            for s in range(12, 16):
                push_v(s)

            # --- main blocks ---
            blocks = [(hp, par, half) for half in (0, 1) for hp, par in HEADS]
            for bi, (hp, par, half) in enumerate(blocks):
                nkcs = 8 if half == 0 else 16
                for kc in range(nkcs):
                    o = 0 if (half == 0 or kc < 8) else 128 * (kc - 8)
                    emit_scores(bi, hp, par, half, kc)
                    if half == 0:
                        if kc == 7:
                            # spread the 8 units across the next block's
                            # periods to avoid a block-boundary burst
                            for qc in range(8):
                                pv_push(
                                    period[0] + 1 + (3 * qc) // 4,
                                    pv_unit(bi, hp, par, half, qc, 8),
                                )
                            pv_push(period[0] + 7, flush_unit(bi, hp, par, half))
                    else:
                        if kc >= 8:
                            qc = kc - 8
                            pv_push(
                                period[0] + 1,
                                pv_unit(bi, hp, par, half, qc, kc + 1),
                            )
                            if kc == 15:
                                pv_push(
                                    period[0] + 1, flush_unit(bi, hp, par, half)
                                )
                    # pace the queue at ~1.05x the exp cadence so ACT (not PE)
                    # absorbs scheduling jitter; scores rows count against it
                    w_ = 1024 - o
                    exp_rows = (0.833 * w_ + 185.0) / 0.4167
                    alloc = 0.93 * exp_rows - w_ // 2
                    if bi >= 6:
                        alloc = max(alloc, 2600.0)
                    pop_rows(alloc)
                    period[0] += 1

            # --- drain ---
            period[0] += 1000
            while pv_q or work:
                while pv_q:
                    _, _, rows, fn = heapq.heappop(pv_q)
                    fn()
                while work:
                    rows, fn = work.popleft()
                    fn()

    nc.compile()
    return nc


_NC = None


def _get_nc():
    global _NC
    if _NC is None:
        _NC = build_nc()
    return _NC


def make_in_maps(x, Wq, bq, Wk, bk, Wv, bv, Wo):
    _get_nc()
    bf = ml_dtypes.bfloat16
    e4 = ml_dtypes.float8_e4m3fn
    x = np.asarray(x, np.float32)
    kk = np.arange(128)[:, None]
    qp = np.arange(128)[None, :]
    tri = (kk <= qp).astype(bf)
    idm = np.eye(128, dtype=np.float32).astype(bf)
    ones = np.ones(512, bf)

    def hilo(a):
        h = a.astype(e4)
        l = (a - h.astype(np.float32)).astype(e4)
        return h, l

    def w8(W, sl):
        # [D, 2, HG]: hi/lo of 16*W
        h, l = hilo(np.asarray(W, np.float32)[:, sl] * 16.0)
        return np.ascontiguousarray(np.stack([h, l], axis=1))

    in_maps = []
    for core in range(8):
        b, g = core // 4, core % 4
        sl = slice(HG * g, HG * (g + 1))
        xh, xl = hilo(x[b].T)
        in_maps.append(
            {
                "xth": np.ascontiguousarray(xh),
                "xtl": np.ascontiguousarray(xl),
                "wq8": w8(Wq, sl),
                "wk8": w8(Wk, sl),
                "wv8": w8(Wv, sl),
                "bq": np.ascontiguousarray(np.asarray(bq, np.float32)[sl]),
                "bv": np.ascontiguousarray(
                    (np.asarray(bv, np.float32)[sl] * 16.0).astype(bf)
                ),
                "wo": np.ascontiguousarray(np.asarray(Wo, np.float32)[sl, :].astype(bf)),
                "tri": tri,
                "idm": idm,
                "ones": ones,
            }
        )
    return in_maps


def kernel(x, Wq, bq, Wk, bk, Wv, bv, Wo, _trace=False, _trace_kwargs=None):
    nc = _get_nc()
    in_maps = make_in_maps(x, Wq, bq, Wk, bk, Wv, bv, Wo)
    res = run_bass_kernel_spmd(
        nc, in_maps, list(range(8)), trace=_trace, **(_trace_kwargs or {})
    )
    out = np.zeros((2, N, D), np.float64)
    for core in range(8):
        out[core // 4] += np.asarray(res.results[core]["y"], np.float64)
    yf = out.astype(np.float32)
    if _trace:
        return yf, res
    return yf


# revision 3
# speedup vs baseline: 1.0151x; 1.0151x over previous
"""Sparse (half-causal) multi-head attention on 8 Trainium2 NeuronCores, v3.

Problem: x[2,2048,1024] -> QKV proj (16 heads, dk=dv=64) -> scores with
half-causal mask (rows <1024 attend cols <1024 dense; rows >=1024 causal)
-> softmax -> out proj.

Sharding: 8 cores = 2 batches x 4 head-groups (4 heads each).  Each core
computes its batch's full QKV for its 4 heads (column-sharded W), attention
for those heads, and a partial output projection (row-sharded Wo).  Host
sums the 4 partials per batch.

v3 changes vs v2 (152.3us -> 125.1us):
 - scores in fp8 e4m3 via DoubleRow perf mode with a stride-0 k-tile slot:
   both DR slots read the same data, computing 2x(k.q) at 0.5 cyc/row; the
   doubling folds into the exp scale (0.0625).  Halves scores PE time.
 - QKV projections as 3-term hi/lo fp8 DoubleRow (xh@Wh + xl@Wh + xh@Wl,
   weights x16 against e4m3 subnormals, 1/16 folded into the psum copy):
   0.75x the bf16 matmul rows at ~bf16 accuracy (rel err 1.5e-2 vs 2e-2
   gate, measured on HW).
 - PV flipped: out[q,v] = probs[k,q].T @ v[k,v] per 128-q chunk, cost 65
   rows/chunk instead of 128 (the 65-wide v+denominator free dim is the
   cheap side).  Denominator becomes a per-partition scalar: recip +
   tensor_scalar multiply, killing v2's dscr broadcast-DMA machinery.
 - att transposed back via PE transpose (identity moving operand) with the
   bf16 output bitcast into the just-consumed pv psum bank (no extra bank).
 - no y2 partial: the causal staircase finishes head (1,1) q-chunk j at
   kc 8+j, so the full out-projection (both head pairs) rides the diagonal.
 - work-queue scheduler paced to the exp cadence: filler (QKV chunks, out-
   projections) pops between scores/exp emissions under a carry-over row
   budget; PV/norm units release one period after their probs so the
   in-order PE stream never stalls on exp; deadline `ensure` pulls keep
   correctness independent of pacing; half-0 out-projections are rationed
   across the half-1 dense blocks to feed PE through the ACT-bound stretch.

Engine budget (TimelineSim): ACT 98.7us (exp-bound), PE 88.0us,
DVE 55us, Pool 29us -> 125.1us total.
"""

import copy as _copy
import heapq
import sys
from collections import deque

if "/opt/trn_rl_repo" not in sys.path:
    sys.path.insert(0, "/opt/trn_rl_repo")

import ml_dtypes
import numpy as np

import concourse.bass as bass  # noqa: F401 (import registers engines)
import concourse.mybir as mybir
import concourse.tile as tile
from concourse import bacc
from concourse.bass_utils import run_bass_kernel_spmd

f32 = mybir.dt.float32
bf16 = mybir.dt.bfloat16
fp8 = mybir.dt.float8e4
AF = mybir.ActivationFunctionType
OP = mybir.AluOpType
PM = mybir.MatmulPerfMode

D = 1024  # d_model
N = 2048  # n_ctx
HG = 256  # head-group width per core (4 heads x 64)

HEADS = [(0, 0), (0, 1), (1, 0), (1, 1)]  # (hp, par)


def stride0(ap):
    """Turn the first singleton non-partition dim into a stride-0 2-count
    dim (the DoubleRow k-tile slot reading the same data twice)."""
    ap2 = _copy.copy(ap)
    lst = ap2.ap
    for i in range(1, len(lst)):
        if lst[i][1] == 1:
            lst[i] = [0, 2]
            return ap2
    raise RuntimeError(f"no singleton dim in {lst}")


def build_nc():
    nc = bacc.Bacc("TRN2", target_bir_lowering=False, debug=False)

    # x and the QKV weights ship as fp8 hi/lo pairs (weights pre-scaled x16
    # on the host; the 1/16 folds into the psum->sbuf copy)
    xth = nc.declare_dram_parameter("xth", [D, N], fp8, isOutput=False)
    xtl = nc.declare_dram_parameter("xtl", [D, N], fp8, isOutput=False)
    wq8 = nc.declare_dram_parameter("wq8", [D, 2, HG], fp8, isOutput=False)
    wk8 = nc.declare_dram_parameter("wk8", [D, 2, HG], fp8, isOutput=False)
    wv8 = nc.declare_dram_parameter("wv8", [D, 2, HG], fp8, isOutput=False)
    bqd = nc.declare_dram_parameter("bq", [HG], f32, isOutput=False)
    bvd = nc.declare_dram_parameter("bv", [HG], bf16, isOutput=False)
    wo = nc.declare_dram_parameter("wo", [HG, D], bf16, isOutput=False)
    trid = nc.declare_dram_parameter("tri", [128, 128], bf16, isOutput=False)
    idmd = nc.declare_dram_parameter("idm", [128, 128], bf16, isOutput=False)
    onesd = nc.declare_dram_parameter("ones", [512], bf16, isOutput=False)
    y = nc.declare_dram_parameter("y", [N, D], bf16, isOutput=True)

    xth_r = xth[:].rearrange("(c p) n -> p c n", p=128)
    xtl_r = xtl[:].rearrange("(c p) n -> p c n", p=128)
    wq_r = wq8[:].rearrange("(c p) hl m -> p c hl m", p=128)
    wk_r = wk8[:].rearrange("(c p) hl m -> p c hl m", p=128)
    wv_r = wv8[:].rearrange("(c p) hl m -> p c hl m", p=128)
    wo_r = wo[:].rearrange("(c p) n -> p c n", p=128)

    with tile.TileContext(nc) as tc:
        with (
            tc.tile_pool(name="persist", bufs=1) as P1,
            tc.tile_pool(name="xtp", bufs=8) as XTP,
            tc.tile_pool(name="ppool", bufs=40) as PP,
            tc.tile_pool(name="aqp", bufs=4) as AQP,
            tc.tile_pool(name="spp", bufs=2) as SPP,
            tc.tile_pool(name="rp", bufs=4) as RP,
            tc.tile_pool(name="yp", bufs=4) as YP,
            tc.tile_pool(name="ps_s", bufs=2, space="PSUM") as PSS,
            tc.tile_pool(name="ps_pv", bufs=2, space="PSUM") as PSPV,
            tc.tile_pool(name="ps_b", bufs=2, space="PSUM") as PSB,
        ):
            # ---------- persistent tiles ----------
            # [part, c-chunk, hi/lo, cols]: the (c, c+1) pairing for DR slots
            # lives in the c dim; hi/lo selects the split
            wq_sb = P1.tile([128, 8, 2, HG], fp8, tag="wq")
            wk_sb = P1.tile([128, 8, 2, HG], fp8, tag="wk")
            wv_sb = P1.tile([128, 8, 2, HG], fp8, tag="wv")
            wo_sb = P1.tile([128, 2, D], bf16, tag="wo")
            bq_sb = P1.tile([128, 2], f32, tag="bq")
            bv_sb = P1.tile([1, HG], bf16, tag="bv")
            ones_sb = P1.tile([1, 512], bf16, tag="ones")
            tri_sb = P1.tile([128, 128], bf16, tag="tri")
            idm_sb = P1.tile([128, 128], bf16, tag="idm")

            ebase = P1.tile([128, 1024], f32, tag="ebase")
            qT8 = P1.tile([128, 2, N], fp8, tag="qT8")
            kT8 = P1.tile([128, 2, N], fp8, tag="kT8")
            v1 = P1.tile([128, 16, 4, 65], bf16, tag="v1")
            att = P1.tile([128, 2, N], bf16, tag="att")

            # ---------- QKV emitters ----------
            xt_tiles = {}

            def emit_load(n4, split=False):
                ns = slice(512 * n4, 512 * n4 + 512)
                a = XTP.tile([128, 8, 512], fp8, tag="xt", name=f"xth{n4}")
                b_ = XTP.tile([128, 8, 512], fp8, tag="xt", name=f"xtl{n4}")
                for h in range(2):
                    cs = slice(4 * h, 4 * h + 4)
                    nc.sync.dma_start(a[:, cs, :], xth_r[:, cs, ns])
                eng = nc.gpsimd if split else nc.sync
                for h in range(2):
                    cs = slice(4 * h, 4 * h + 4)
                    eng.dma_start(b_[:, cs, :], xtl_r[:, cs, ns])
                xt_tiles[n4] = (a, b_)

            def emit_qk_part(which, n4, m, toff, twid, act_copy=False):
                """Q/K projection for a token sub-range via 3-term hi/lo fp8
                DoubleRow (xh@Wh + xl@Wh + xh@Wl); weights are x16 so the
                psum->sbuf copy scales by 1/16 (and adds bq for Q)."""
                ns = slice(512 * n4 + toff, 512 * n4 + toff + twid)
                xh, xl = xt_tiles[n4]
                msl = slice(128 * m, 128 * m + 128)
                w_sb = wq_sb if which == "q" else wk_sb
                ps = PSB.tile(
                    [128, 512], f32, tag="b", name=f"{which}ps{n4}{m}{toff}"
                )
                terms = [(xh, 0), (xl, 0), (xh, 1)]
                for ti, (xs, hl) in enumerate(terms):
                    for j in range(4):
                        nc.tensor.matmul(
                            ps[:, 0:twid],
                            w_sb[:, 2 * j : 2 * j + 2, hl, msl],
                            xs[:, 2 * j : 2 * j + 2, toff : toff + twid],
                            start=(ti == 0 and j == 0),
                            stop=(ti == 2 and j == 3),
                            perf_mode=PM.DoubleRow,
                        )
                if which == "q":
                    with nc.allow_low_precision(reason="fp8 qT"):
                        if act_copy:
                            nc.scalar.activation(
                                qT8[:, m, ns], ps[:, 0:twid], AF.Identity,
                                bias=bq_sb[:, m : m + 1], scale=1.0 / 16.0,
                            )
                        else:
                            nc.vector.tensor_scalar(
                                out=qT8[:, m, ns], in0=ps[:, 0:twid],
                                scalar1=1.0 / 16.0,
                                scalar2=bq_sb[:, m : m + 1],
                                op0=OP.mult, op1=OP.add,
                            )
                else:
                    with nc.allow_low_precision(reason="fp8 kT"):
                        if act_copy:
                            nc.scalar.mul(kT8[:, m, ns], ps[:, 0:twid], 1.0 / 16.0)
                        else:
                            nc.vector.tensor_scalar(
                                out=kT8[:, m, ns], in0=ps[:, 0:twid],
                                scalar1=1.0 / 16.0, scalar2=None, op0=OP.mult,
                            )

            def emit_q(n4, m, act_copy=False):
                emit_qk_part("q", n4, m, 0, 512, act_copy=act_copy)

            def emit_k(n4, m, act_copy=False):
                emit_qk_part("k", n4, m, 0, 512, act_copy=act_copy)

            def emit_v(s, act_copy=False):
                n4 = s // 4
                xh, xl = xt_tiles[n4]
                so = 128 * (s - 4 * n4)
                ps = PSB.tile([128, 256], f32, tag="b", name=f"vps{s}")
                terms = [(xh, 0), (xl, 0), (xh, 1)]
                for ti, (xs, hl) in enumerate(terms):
                    for j in range(4):
                        nc.tensor.matmul(
                            ps[:],
                            xs[:, 2 * j : 2 * j + 2, so : so + 128],
                            wv_sb[:, 2 * j : 2 * j + 2, hl, :],
                            start=(ti == 0 and j == 0),
                            stop=False,
                            perf_mode=PM.DoubleRow,
                        )
                # bias (x16 on host, like the weights)
                nc.tensor.matmul(
                    ps[:], ones_sb[:, :128], bv_sb[:], start=False, stop=True
                )
                with nc.allow_low_precision(reason="bf16 v1"):
                    src = ps[:].rearrange("p (h d) -> p h d", h=4)
                    if act_copy:
                        nc.scalar.mul(v1[:, s, :, 0:64], src, 1.0 / 16.0)
                    else:
                        nc.vector.tensor_scalar(
                            out=v1[:, s, :, 0:64], in0=src,
                            scalar1=1.0 / 16.0, scalar2=None, op0=OP.mult,
                        )

            # ---------- attention ----------
            probs = {}  # (bi, kc) -> p_t

            sc_tiles = {}

            def emit_scores_piece(bi, hp, par, half, kc, lo, hi):
                """One <=512-wide scores matmul + its exp piece."""
                q0 = 1024 * half
                base = 64 * par
                ensure(("q", 2 * half + lo // 512, hp, 0))
                ensure(("q", 2 * half + lo // 512, hp, 256))
                if hi - lo > 512 - 256:  # piece spans into the next q-256
                    ensure(("q", 2 * half + (hi - 1) // 512, hp, 0))
                    ensure(("q", 2 * half + (hi - 1) // 512, hp, 256))
                ensure(("k", kc // 4, hp, 256 * ((kc % 4) // 2)))
                s_t, p_t = sc_tiles[(bi, kc)]
                lhsT = stride0(kT8[base : base + 64, hp : hp + 1,
                                   128 * kc : 128 * kc + 128])
                rhs = stride0(qT8[base : base + 64, hp : hp + 1,
                                  q0 + lo : q0 + hi])
                nc.tensor.matmul(
                    s_t[:, lo:hi], lhsT, rhs, start=True, stop=True,
                    perf_mode=PM.DoubleRow,
                )
                with nc.allow_low_precision(reason="bf16 probs"):
                    # DR stride-0 doubles the score; 1/16 = 0.5 * 1/8
                    nc.scalar.activation(
                        p_t[:, lo:hi], s_t[:, lo:hi], AF.Exp, scale=0.0625
                    )

            def emit_scores(bi, hp, par, half, kc, split=False):
                q0 = 1024 * half
                base = 64 * par
                o = 0 if (half == 0 or kc < 8) else 128 * (kc - 8)
                s_t = PSS.tile([128, 1024], f32, tag="s", name=f"s{bi}_{kc}")
                p_t = PP.tile([128, 1024], bf16, tag="p", name=f"p{bi}_{kc}")
                sc_tiles[(bi, kc)] = (s_t, p_t)
                probs[(bi, kc)] = p_t
                if split:
                    # B0 ramp compression: emit only the lo half now; the hi
                    # half (needing q(1,0)) is emitted via emit_scores_piece
                    emit_scores_piece(bi, hp, par, half, kc, 0, 512)
                    return
                # correctness guards: prerequisite projections must be emitted
                for n4 in (2 * half, 2 * half + 1):
                    for toff in (0, 256):
                        ensure(("q", n4, hp, toff))
                ensure(("k", kc // 4, hp, 256 * ((kc % 4) // 2)))
                lhsT = stride0(kT8[base : base + 64, hp : hp + 1,
                                   128 * kc : 128 * kc + 128])
                # matmul out must stay within one PSUM bank (<=512 f32)
                mm = [(o, 512), (512, 1024)] if o < 512 else [(o, 1024)]
                for lo, hi in mm:
                    rhs = stride0(qT8[base : base + 64, hp : hp + 1,
                                      q0 + lo : q0 + hi])
                    nc.tensor.matmul(
                        s_t[:, lo:hi], lhsT, rhs, start=True, stop=True,
                        perf_mode=PM.DoubleRow,
                    )
                w_ = 1024 - o
                if half == 1 and kc <= -1:
                    # offload early dense exps (their probs are consumed ~8
                    # periods later, hiding the DVE->Pool chain latency):
                    # DVE stages scores to SBUF, Pool computes base^s
                    s_sb = SPP.tile(
                        [128, 1024], f32, tag="ss", name=f"ss{bi}_{kc}"
                    )
                    nc.vector.tensor_copy(out=s_sb[:, 0:w_], in_=s_t[:, o:1024])
                    with nc.allow_low_precision(reason="bf16 probs"):
                        nc.gpsimd.tensor_tensor(
                            p_t[:, o:1024], ebase[:, 0:w_], s_sb[:, 0:w_],
                            OP.pow,
                        )
                else:
                    with nc.allow_low_precision(reason="bf16 probs"):
                        # DR stride-0 doubles the score; 1/16 = 0.5 * 1/8
                        nc.scalar.activation(
                            p_t[:, o:1024], s_t[:, o:1024], AF.Exp, scale=0.0625
                        )
                if half == 1 and kc >= 8:
                    # causal mask inside the diagonal 128-block. Pool for the
                    # early blocks; DVE for the last two, whose staircase
                    # would otherwise queue behind slow SWDGE y-DMAs on Pool
                    eng = nc.gpsimd
                    eng.tensor_tensor(
                        p_t[:, o : o + 128], p_t[:, o : o + 128], tri_sb[:],
                        OP.mult,
                    )
                probs[(bi, kc)] = p_t

            # per-block deferred transpose state: (pv, attq, qc)
            pend_fin = {}

            def emit_pv(bi, hp, par, half, qc, nkc):
                for kc in range(nkc):
                    ensure(("v", kc))
                h2 = 2 * hp + par
                pv = PSPV.tile([128, 512], f32, tag="pv", name=f"pv{bi}_{qc}")
                col = slice(128 * qc, 128 * qc + 128)
                for kc in range(nkc):
                    nc.tensor.matmul(
                        pv[:, 0:65],
                        probs[(bi, kc)][:, col],
                        v1[:, kc, h2, :],
                        start=(kc == 0),
                        stop=(kc == nkc - 1),
                    )
                return pv

            def emit_norm(bi, qc, pv, act=False):
                r = RP.tile([128, 1], f32, tag="r", name=f"r{bi}_{qc}")
                nc.vector.reciprocal(r[:], pv[:, 64:65])
                attq = AQP.tile([128, 64], bf16, tag="aq", name=f"aq{bi}_{qc}")
                with nc.allow_low_precision(reason="bf16 attq"):
                    if act:
                        # ACT is idle in the tail: out = in * r (per-partition)
                        nc.scalar.activation(
                            attq[:], pv[:, 0:64], AF.Copy, scale=r[:]
                        )
                    else:
                        nc.vector.tensor_scalar(
                            out=attq[:], in0=pv[:, 0:64], scalar1=r[:],
                            scalar2=None, op0=OP.mult,
                        )
                return attq

            def emit_finish(hp, par, half, qc, pv, attq, act=False):
                # transpose attq -> [64,128] into the dead pv bank, then copy
                base = 64 * par
                q0 = 1024 * half
                trout = pv[base : base + 64, 128:192].bitcast(bf16)
                nc.tensor.matmul(
                    trout, attq[:], idm_sb[:], start=True, stop=True,
                    is_transpose=True,
                )
                with nc.allow_low_precision(reason="bf16 att"):
                    dst = att[base : base + 64, hp,
                              q0 + 128 * qc : q0 + 128 * qc + 128]
                    if act:
                        nc.scalar.copy(out=dst, in_=trout)
                    else:
                        nc.vector.tensor_copy(out=dst, in_=trout)

            # ---------- output projection ----------
            yt_tiles = {}

            def emit_outproj_seg(s, nseg, late=False):
                if nseg == 0:
                    yt_tiles[s] = YP.tile([128, D], bf16, tag="y", name=f"yt{s}")
                yt = yt_tiles[s]
                ps = PSB.tile([128, 512], f32, tag="b", name=f"yps{s}_{nseg}")
                for hp in range(2):
                    nc.tensor.matmul(
                        ps[:],
                        att[:, hp, 128 * s : 128 * s + 128],
                        wo_sb[:, hp, 512 * nseg : 512 * nseg + 512],
                        start=(hp == 0),
                        stop=(hp == 1),
                    )
                sl = slice(512 * nseg, 512 * nseg + 512)
                with nc.allow_low_precision(reason="bf16 y"):
                    if s >= 13:
                        nc.scalar.copy(out=yt[:, sl], in_=ps[:])
                    else:
                        nc.vector.tensor_copy(out=yt[:, sl], in_=ps[:])
                if nseg == 1:
                    del yt_tiles[s]
                    # dual-queue issue so the sequencer cost doesn't serialize
                    # the drain; keep the slower SWDGE path off the last chunks
                    eng = nc.gpsimd if not late else nc.sync
                    eng.dma_start(y[128 * s : 128 * s + 128, :], yt[:])

            # ============================================================
            # work queue
            # ============================================================
            work = deque()  # (rows, fn, key) — filler units
            done = set()  # keys of emitted filler units
            pv_q = []  # heap of (release_period, seq, rows, fn)
            pv_seq = [0]
            period = [0]
            acct = [0.0]  # carry-over PE-row budget

            def run_unit(rows, fn, key):
                if key is not None:
                    if key in done:
                        return 0
                    done.add(key)
                fn()
                return rows

            def ensure(key):
                """Emit a queued filler unit NOW if it hasn't run yet —
                correctness guard so pacing can never reorder a consumer
                ahead of its producer."""
                if key in done:
                    return
                for i, (rows, fn, k) in enumerate(work):
                    if k == key:
                        del work[i]
                        run_unit(rows, fn, key)
                        acct[0] -= rows
                        return
                raise RuntimeError(f"missing unit {key}")

            def pv_push(release, rows_fn):
                rows, fn = rows_fn
                heapq.heappush(pv_q, (release, pv_seq[0], rows, fn))
                pv_seq[0] += 1

            def pop_rows(alloc):
                # released PV/op units first (their release period guarantees
                # deps are done, so they never stall the in-order PE stream),
                # then ready filler while the carry-over account affords it
                acct[0] = min(acct[0] + alloc, 1.5 * alloc if alloc > 0 else acct[0])
                while pv_q and pv_q[0][0] <= period[0]:
                    _, _, rows, fn = heapq.heappop(pv_q)
                    fn()
                    acct[0] -= rows
                while work and work[0][0] <= acct[0]:
                    rows, fn, key = work.popleft()
                    acct[0] -= run_unit(rows, fn, key)

            def pv_unit(bi, hp, par, half, qc, nkc):
                act = bi >= 7

                def fn():
                    pv = emit_pv(bi, hp, par, half, qc, nkc)
                    attq = emit_norm(bi, qc, pv, act=act)
                    if bi in pend_fin:
                        ppv, pattq, pqc = pend_fin.pop(bi)
                        emit_finish(hp, par, half, pqc, ppv, pattq, act=act)
                        after_finish(bi, half, pqc)
                    pend_fin[bi] = (pv, attq, qc)

                return (65 * nkc + 192, fn)

            def flush_unit(bi, hp, par, half):
                act = bi >= 7

                def fn():
                    if bi in pend_fin:
                        ppv, pattq, pqc = pend_fin.pop(bi)
                        emit_finish(hp, par, half, pqc, ppv, pattq, act=act)
                        after_finish(bi, half, pqc)

                return (192, fn)

            def after_finish(bi, half, qc):
                # out-projections unlock when the LAST head (block bi=3 for
                # half 0, bi=7 for half 1) lands its att chunk
                if bi == 3 and qc == 7:
                    # ration the deadline-free half-0 out-projections across
                    # blocks 4-6 so the diag stretches keep PE fed
                    for i in range(16):
                        s, nseg = i // 2, i % 2
                        pv_push(
                            period[0] + 2 + 3 * i,
                            (1024, lambda s=s, n=nseg: emit_outproj_seg(s, n)),
                        )
                elif bi == 7:
                    s = 8 + qc
                    for nseg in range(2):
                        pv_push(
                            period[0],
                            (1024, lambda s=s, n=nseg:
                             emit_outproj_seg(s, n, late=(s >= 12))),
                        )

            # ============================================================
            # emission schedule
            # ============================================================
            # --- ramp: stream x/weights, QKV chunks 0-1 ---
            nc.gpsimd.memzero(ones_sb[:])
            with nc.allow_low_precision(reason="bf16 ones"):
                nc.gpsimd.tensor_scalar_add(ones_sb[:], ones_sb[:], 1.0)
            nc.gpsimd.memset(ebase[:], float(np.exp(0.0625)))
            nc.scalar.dma_start(wq_sb[:, 0:4, :, :], wq_r[:, 0:4, :, :])
            emit_load(0, split=True)
            nc.scalar.dma_start(wq_sb[:, 4:8, :, :], wq_r[:, 4:8, :, :])
            nc.scalar.dma_start(wk_sb[:, 0:4, :, :], wk_r[:, 0:4, :, :])
            nc.scalar.dma_start(wk_sb[:, 4:8, :, :], wk_r[:, 4:8, :, :])
            nc.scalar.dma_start(wv_sb[:], wv_r[:])
            nc.gpsimd.dma_start(bq_sb[:], bqd[:].rearrange("(m p) -> p m", p=128))
            nc.gpsimd.dma_start(bv_sb[:], bvd[None, :])
            nc.scalar.dma_start(tri_sb[:], trid[:])
            nc.scalar.dma_start(idm_sb[:], idmd[:])
            # warmup matmuls: climb the PE p-state while DMAs stream
            for w in range(8):
                junk = PSB.tile([128, 512], f32, tag="b", name=f"warm{w}")
                nc.tensor.matmul(
                    junk[:], ones_sb[0:1, :128], ones_sb[:], start=True, stop=True
                )
            # minimal pre-B0 projections: only q(0,0) and k(0,0); B0 emits
            # 512-wide split scores so exp starts before q(1,0) exists
            emit_q(0, 0, act_copy=True)
            emit_k(0, 0, act_copy=True)
            emit_load(1)
            emit_q(1, 0, act_copy=True)
            nc.sync.dma_start(v1[:, :, :, 64:65], onesd[0:64].partition_broadcast(128))
            emit_load(2)
            emit_load(3)
            nc.scalar.dma_start(wo_sb[:], wo_r[:])

            # ramp already emitted these:
            done.update({("q", 0, 0, 0), ("q", 0, 0, 256),
                         ("k", 0, 0, 0), ("k", 0, 0, 256),
                         ("q", 1, 0, 0), ("q", 1, 0, 256)})

            def push_qk(which, n4, m):
                for toff in (0, 256):
                    work.append(
                        (1536,
                         lambda n4=n4, m=m, toff=toff:
                         emit_qk_part(which, n4, m, toff, 256),
                         (which, n4, m, toff))
                    )

            # --- remaining QKV queued as filler, in deadline order ---
            # v0-7 + k(1,0): B0; m=1 units: B2 kc0; q n4 2-3: B4; v8-15: B5
            def push_v(s):
                work.append((1792, lambda s=s: emit_v(s), ("v", s)))

            push_v(0)
            push_v(1)
            push_qk("k", 1, 0)
            for s in range(2, 8):
                push_v(s)
            push_qk("q", 0, 1)
            push_qk("k", 0, 1)
            push_qk("q", 1, 1)
            push_qk("k", 1, 1)
            push_qk("q", 2, 0)
            push_qk("q", 3, 0)
            push_qk("q", 2, 1)
            push_qk("q", 3, 1)
            push_qk("k", 2, 0)
            push_qk("k", 3, 0)
            for s in range(8, 12):
                push_v(s)
            push_qk("k", 2, 1)
            push_qk("k", 3, 1)
            for s in range(12, 16):
                push_v(s)

            # --- main blocks ---
            blocks = [(hp, par, half) for half in (0, 1) for hp, par in HEADS]
            for bi, (hp, par, half) in enumerate(blocks):
                nkcs = 8 if half == 0 else 16
                for kc in range(nkcs):
                    o = 0 if (half == 0 or kc < 8) else 128 * (kc - 8)
                    emit_scores(bi, hp, par, half, kc)
                    if half == 0:
                        if kc == 7:
                            # spread the 8 units across the next block's
                            # periods to avoid a block-boundary burst
                            for qc in range(8):
                                pv_push(
                                    period[0] + 1 + (3 * qc) // 4,
                                    pv_unit(bi, hp, par, half, qc, 8),
                                )
                            pv_push(period[0] + 7, flush_unit(bi, hp, par, half))
                    else:
                        if kc >= 8:
                            qc = kc - 8
                            pv_push(
                                period[0] + 1,
                                pv_unit(bi, hp, par, half, qc, kc + 1),
                            )
                            if kc == 15:
                                pv_push(
                                    period[0] + 1, flush_unit(bi, hp, par, half)
                                )
                    # pace the queue at ~1.05x the exp cadence so ACT (not PE)
                    # absorbs scheduling jitter; scores rows count against it
                    w_ = 1024 - o
                    exp_rows = (0.833 * w_ + 185.0) / 0.4167
                    alloc = 0.93 * exp_rows - w_ // 2
                    if bi >= 6:
                        alloc = max(alloc, 3400.0)
                    pop_rows(alloc)
                    period[0] += 1

            # --- drain ---
            period[0] += 1000
            while pv_q or work:
                while pv_q:
                    _, _, rows, fn = heapq.heappop(pv_q)
                    fn()
                while work:
                    rows, fn = work.popleft()
                    fn()

    nc.compile()
    return nc


_NC = None


def _get_nc():
    global _NC
    if _NC is None:
        _NC = build_nc()
    return _NC


def make_in_maps(x, Wq, bq, Wk, bk, Wv, bv, Wo):
    _get_nc()
    bf = ml_dtypes.bfloat16
    e4 = ml_dtypes.float8_e4m3fn
    x = np.asarray(x, np.float32)
    kk = np.arange(128)[:, None]
    qp = np.arange(128)[None, :]
    tri = (kk <= qp).astype(bf)
    idm = np.eye(128, dtype=np.float32).astype(bf)
    ones = np.ones(512, bf)

    def hilo(a):
        h = a.astype(e4)
        l = (a - h.astype(np.float32)).astype(e4)
        return h, l

    def w8(W, sl):
        # [D, 2, HG]: hi/lo of 16*W
        h, l = hilo(np.asarray(W, np.float32)[:, sl] * 16.0)
        return np.ascontiguousarray(np.stack([h, l], axis=1))

    in_maps = []
    for core in range(8):
        b, g = core // 4, core % 4
        sl = slice(HG * g, HG * (g + 1))
        xh, xl = hilo(x[b].T)
        in_maps.append(
            {
                "xth": np.ascontiguousarray(xh),
                "xtl": np.ascontiguousarray(xl),
                "wq8": w8(Wq, sl),
                "wk8": w8(Wk, sl),
                "wv8": w8(Wv, sl),
                "bq": np.ascontiguousarray(np.asarray(bq, np.float32)[sl]),
                "bv": np.ascontiguousarray(
                    (np.asarray(bv, np.float32)[sl] * 16.0).astype(bf)
                ),
                "wo": np.ascontiguousarray(np.asarray(Wo, np.float32)[sl, :].astype(bf)),
                "tri": tri,
                "idm": idm,
                "ones": ones,
            }
        )
    return in_maps


def kernel(x, Wq, bq, Wk, bk, Wv, bv, Wo, _trace=False, _trace_kwargs=None):
    nc = _get_nc()
    in_maps = make_in_maps(x, Wq, bq, Wk, bk, Wv, bv, Wo)
    res = run_bass_kernel_spmd(
        nc, in_maps, list(range(8)), trace=_trace, **(_trace_kwargs or {})
    )
    out = np.zeros((2, N, D), np.float64)
    for core in range(8):
        out[core // 4] += np.asarray(res.results[core]["y"], np.float64)
    yf = out.astype(np.float32)
    if _trace:
        return yf, res
    return yf


# revision 4
# speedup vs baseline: 1.0197x; 1.0045x over previous
"""Sparse (half-causal) multi-head attention on 8 Trainium2 NeuronCores, v3.

Problem: x[2,2048,1024] -> QKV proj (16 heads, dk=dv=64) -> scores with
half-causal mask (rows <1024 attend cols <1024 dense; rows >=1024 causal)
-> softmax -> out proj.

Sharding: 8 cores = 2 batches x 4 head-groups (4 heads each).  Each core
computes its batch's full QKV for its 4 heads (column-sharded W), attention
for those heads, and a partial output projection (row-sharded Wo).  Host
sums the 4 partials per batch.

v3 changes vs v2 (152.3us -> 123.3us):
 - scores in fp8 e4m3 via DoubleRow perf mode with a stride-0 k-tile slot:
   both DR slots read the same data, computing 2x(k.q) at 0.5 cyc/row; the
   doubling folds into the exp scale (0.0625).  Halves scores PE time.
 - QKV projections as 3-term hi/lo fp8 DoubleRow (xh@Wh + xl@Wh + xh@Wl,
   weights x16 against e4m3 subnormals, 1/16 folded into the psum copy):
   0.75x the bf16 matmul rows at ~bf16 accuracy (rel err 1.5e-2 vs 2e-2
   gate, measured on HW).
 - PV flipped: out[q,v] = probs[k,q].T @ v[k,v] per 128-q chunk, cost 65
   rows/chunk instead of 128 (the 65-wide v+denominator free dim is the
   cheap side).  Denominator becomes a per-partition scalar: recip +
   tensor_scalar multiply, killing v2's dscr broadcast-DMA machinery.
 - att transposed back via PE transpose (identity moving operand) with the
   bf16 output bitcast into the just-consumed pv psum bank (no extra bank).
 - no y2 partial: the causal staircase finishes head (1,1) q-chunk j at
   kc 8+j, so the full out-projection (both head pairs) rides the diagonal.
 - work-queue scheduler paced to the exp cadence: filler (QKV chunks, out-
   projections) pops between scores/exp emissions under a carry-over row
   budget; PV/norm units release one period after their probs so the
   in-order PE stream never stalls on exp; deadline `ensure` pulls keep
   correctness independent of pacing; half-0 out-projections are rationed
   across the half-1 dense blocks to feed PE through the ACT-bound stretch.

Engine budget (TimelineSim): ACT 98.7us (exp-bound), PE 88.0us,
DVE 55us, Pool 29us -> 123.3us total (drain copies alternate ACT/DVE;
the first two kc's lo-half scores fire in the ramp).
"""

import copy as _copy
import heapq
import sys
from collections import deque

if "/opt/trn_rl_repo" not in sys.path:
    sys.path.insert(0, "/opt/trn_rl_repo")

import ml_dtypes
import numpy as np

import concourse.bass as bass  # noqa: F401 (import registers engines)
import concourse.mybir as mybir
import concourse.tile as tile
from concourse import bacc
from concourse.bass_utils import run_bass_kernel_spmd

f32 = mybir.dt.float32
bf16 = mybir.dt.bfloat16
fp8 = mybir.dt.float8e4
AF = mybir.ActivationFunctionType
OP = mybir.AluOpType
PM = mybir.MatmulPerfMode

D = 1024  # d_model
N = 2048  # n_ctx
HG = 256  # head-group width per core (4 heads x 64)

HEADS = [(0, 0), (0, 1), (1, 0), (1, 1)]  # (hp, par)


def stride0(ap):
    """Turn the first singleton non-partition dim into a stride-0 2-count
    dim (the DoubleRow k-tile slot reading the same data twice)."""
    ap2 = _copy.copy(ap)
    lst = ap2.ap
    for i in range(1, len(lst)):
        if lst[i][1] == 1:
            lst[i] = [0, 2]
            return ap2
    raise RuntimeError(f"no singleton dim in {lst}")


def build_nc():
    nc = bacc.Bacc("TRN2", target_bir_lowering=False, debug=False)

    # x and the QKV weights ship as fp8 hi/lo pairs (weights pre-scaled x16
    # on the host; the 1/16 folds into the psum->sbuf copy)
    xth = nc.declare_dram_parameter("xth", [D, N], fp8, isOutput=False)
    xtl = nc.declare_dram_parameter("xtl", [D, N], fp8, isOutput=False)
    wq8 = nc.declare_dram_parameter("wq8", [D, 2, HG], fp8, isOutput=False)
    wk8 = nc.declare_dram_parameter("wk8", [D, 2, HG], fp8, isOutput=False)
    wv8 = nc.declare_dram_parameter("wv8", [D, 2, HG], fp8, isOutput=False)
    bqd = nc.declare_dram_parameter("bq", [HG], f32, isOutput=False)
    bvd = nc.declare_dram_parameter("bv", [HG], bf16, isOutput=False)
    wo = nc.declare_dram_parameter("wo", [HG, D], bf16, isOutput=False)
    trid = nc.declare_dram_parameter("tri", [128, 128], bf16, isOutput=False)
    idmd = nc.declare_dram_parameter("idm", [128, 128], bf16, isOutput=False)
    onesd = nc.declare_dram_parameter("ones", [512], bf16, isOutput=False)
    y = nc.declare_dram_parameter("y", [N, D], bf16, isOutput=True)

    xth_r = xth[:].rearrange("(c p) n -> p c n", p=128)
    xtl_r = xtl[:].rearrange("(c p) n -> p c n", p=128)
    wq_r = wq8[:].rearrange("(c p) hl m -> p c hl m", p=128)
    wk_r = wk8[:].rearrange("(c p) hl m -> p c hl m", p=128)
    wv_r = wv8[:].rearrange("(c p) hl m -> p c hl m", p=128)
    wo_r = wo[:].rearrange("(c p) n -> p c n", p=128)

    with tile.TileContext(nc) as tc:
        with (
            tc.tile_pool(name="persist", bufs=1) as P1,
            tc.tile_pool(name="xtp", bufs=8) as XTP,
            tc.tile_pool(name="ppool", bufs=40) as PP,
            tc.tile_pool(name="aqp", bufs=4) as AQP,
            tc.tile_pool(name="spp", bufs=2) as SPP,
            tc.tile_pool(name="rp", bufs=4) as RP,
            tc.tile_pool(name="yp", bufs=4) as YP,
            tc.tile_pool(name="ps_s", bufs=2, space="PSUM") as PSS,
            tc.tile_pool(name="ps_pv", bufs=2, space="PSUM") as PSPV,
            tc.tile_pool(name="ps_b", bufs=2, space="PSUM") as PSB,
        ):
            # ---------- persistent tiles ----------
            # [part, c-chunk, hi/lo, cols]: the (c, c+1) pairing for DR slots
            # lives in the c dim; hi/lo selects the split
            wq_sb = P1.tile([128, 8, 2, HG], fp8, tag="wq")
            wk_sb = P1.tile([128, 8, 2, HG], fp8, tag="wk")
            wv_sb = P1.tile([128, 8, 2, HG], fp8, tag="wv")
            wo_sb = P1.tile([128, 2, D], bf16, tag="wo")
            bq_sb = P1.tile([128, 2], f32, tag="bq")
            bv_sb = P1.tile([1, HG], bf16, tag="bv")
            ones_sb = P1.tile([1, 512], bf16, tag="ones")
            tri_sb = P1.tile([128, 128], bf16, tag="tri")
            idm_sb = P1.tile([128, 128], bf16, tag="idm")

            ebase = P1.tile([128, 1024], f32, tag="ebase")
            qT8 = P1.tile([128, 2, N], fp8, tag="qT8")
            kT8 = P1.tile([128, 2, N], fp8, tag="kT8")
            v1 = P1.tile([128, 16, 4, 65], bf16, tag="v1")
            att = P1.tile([128, 2, N], bf16, tag="att")

            # ---------- QKV emitters ----------
            xt_tiles = {}

            def emit_load(n4, split=False):
                ns = slice(512 * n4, 512 * n4 + 512)
                a = XTP.tile([128, 8, 512], fp8, tag="xt", name=f"xth{n4}")
                b_ = XTP.tile([128, 8, 512], fp8, tag="xt", name=f"xtl{n4}")
                for h in range(2):
                    cs = slice(4 * h, 4 * h + 4)
                    nc.sync.dma_start(a[:, cs, :], xth_r[:, cs, ns])
                eng = nc.gpsimd if split else nc.sync
                for h in range(2):
                    cs = slice(4 * h, 4 * h + 4)
                    eng.dma_start(b_[:, cs, :], xtl_r[:, cs, ns])
                xt_tiles[n4] = (a, b_)

            def emit_qk_part(which, n4, m, toff, twid, act_copy=False):
                """Q/K projection for a token sub-range via 3-term hi/lo fp8
                DoubleRow (xh@Wh + xl@Wh + xh@Wl); weights are x16 so the
                psum->sbuf copy scales by 1/16 (and adds bq for Q)."""
                ns = slice(512 * n4 + toff, 512 * n4 + toff + twid)
                xh, xl = xt_tiles[n4]
                msl = slice(128 * m, 128 * m + 128)
                w_sb = wq_sb if which == "q" else wk_sb
                ps = PSB.tile(
                    [128, 512], f32, tag="b", name=f"{which}ps{n4}{m}{toff}"
                )
                terms = [(xh, 0), (xl, 0), (xh, 1)]
                for ti, (xs, hl) in enumerate(terms):
                    for j in range(4):
                        nc.tensor.matmul(
                            ps[:, 0:twid],
                            w_sb[:, 2 * j : 2 * j + 2, hl, msl],
                            xs[:, 2 * j : 2 * j + 2, toff : toff + twid],
                            start=(ti == 0 and j == 0),
                            stop=(ti == 2 and j == 3),
                            perf_mode=PM.DoubleRow,
                        )
                if which == "q":
                    with nc.allow_low_precision(reason="fp8 qT"):
                        if act_copy:
                            nc.scalar.activation(
                                qT8[:, m, ns], ps[:, 0:twid], AF.Identity,
                                bias=bq_sb[:, m : m + 1], scale=1.0 / 16.0,
                            )
                        else:
                            nc.vector.tensor_scalar(
                                out=qT8[:, m, ns], in0=ps[:, 0:twid],
                                scalar1=1.0 / 16.0,
                                scalar2=bq_sb[:, m : m + 1],
                                op0=OP.mult, op1=OP.add,
                            )
                else:
                    with nc.allow_low_precision(reason="fp8 kT"):
                        if act_copy:
                            nc.scalar.mul(kT8[:, m, ns], ps[:, 0:twid], 1.0 / 16.0)
                        else:
                            nc.vector.tensor_scalar(
                                out=kT8[:, m, ns], in0=ps[:, 0:twid],
                                scalar1=1.0 / 16.0, scalar2=None, op0=OP.mult,
                            )

            def emit_q(n4, m, act_copy=False):
                emit_qk_part("q", n4, m, 0, 512, act_copy=act_copy)

            def emit_k(n4, m, act_copy=False):
                emit_qk_part("k", n4, m, 0, 512, act_copy=act_copy)

            def emit_v(s, act_copy=False):
                n4 = s // 4
                xh, xl = xt_tiles[n4]
                so = 128 * (s - 4 * n4)
                ps = PSB.tile([128, 256], f32, tag="b", name=f"vps{s}")
                terms = [(xh, 0), (xl, 0), (xh, 1)]
                for ti, (xs, hl) in enumerate(terms):
                    for j in range(4):
                        nc.tensor.matmul(
                            ps[:],
                            xs[:, 2 * j : 2 * j + 2, so : so + 128],
                            wv_sb[:, 2 * j : 2 * j + 2, hl, :],
                            start=(ti == 0 and j == 0),
                            stop=False,
                            perf_mode=PM.DoubleRow,
                        )
                # bias (x16 on host, like the weights)
                nc.tensor.matmul(
                    ps[:], ones_sb[:, :128], bv_sb[:], start=False, stop=True
                )
                with nc.allow_low_precision(reason="bf16 v1"):
                    src = ps[:].rearrange("p (h d) -> p h d", h=4)
                    if act_copy:
                        nc.scalar.mul(v1[:, s, :, 0:64], src, 1.0 / 16.0)
                    else:
                        nc.vector.tensor_scalar(
                            out=v1[:, s, :, 0:64], in0=src,
                            scalar1=1.0 / 16.0, scalar2=None, op0=OP.mult,
                        )

            # ---------- attention ----------
            probs = {}  # (bi, kc) -> p_t

            sc_tiles = {}

            def emit_scores_piece(bi, hp, par, half, kc, lo, hi):
                """One <=512-wide scores matmul + its exp piece."""
                q0 = 1024 * half
                base = 64 * par
                ensure(("q", 2 * half + lo // 512, hp, 0))
                ensure(("q", 2 * half + lo // 512, hp, 256))
                if hi - lo > 512 - 256:  # piece spans into the next q-256
                    ensure(("q", 2 * half + (hi - 1) // 512, hp, 0))
                    ensure(("q", 2 * half + (hi - 1) // 512, hp, 256))
                ensure(("k", kc // 4, hp, 256 * ((kc % 4) // 2)))
                s_t, p_t = sc_tiles[(bi, kc)]
                lhsT = stride0(kT8[base : base + 64, hp : hp + 1,
                                   128 * kc : 128 * kc + 128])
                rhs = stride0(qT8[base : base + 64, hp : hp + 1,
                                  q0 + lo : q0 + hi])
                nc.tensor.matmul(
                    s_t[:, lo:hi], lhsT, rhs, start=True, stop=True,
                    perf_mode=PM.DoubleRow,
                )
                with nc.allow_low_precision(reason="bf16 probs"):
                    # DR stride-0 doubles the score; 1/16 = 0.5 * 1/8
                    nc.scalar.activation(
                        p_t[:, lo:hi], s_t[:, lo:hi], AF.Exp, scale=0.0625
                    )

            def emit_scores(bi, hp, par, half, kc, split=False):
                q0 = 1024 * half
                base = 64 * par
                o = 0 if (half == 0 or kc < 8) else 128 * (kc - 8)
                s_t = PSS.tile([128, 1024], f32, tag="s", name=f"s{bi}_{kc}")
                p_t = PP.tile([128, 1024], bf16, tag="p", name=f"p{bi}_{kc}")
                sc_tiles[(bi, kc)] = (s_t, p_t)
                probs[(bi, kc)] = p_t
                if split:
                    # B0 ramp compression: emit only the lo half now; the hi
                    # half (needing q(1,0)) is emitted via emit_scores_piece
                    emit_scores_piece(bi, hp, par, half, kc, 0, 512)
                    return
                # correctness guards: prerequisite projections must be emitted
                for n4 in (2 * half, 2 * half + 1):
                    for toff in (0, 256):
                        ensure(("q", n4, hp, toff))
                ensure(("k", kc // 4, hp, 256 * ((kc % 4) // 2)))
                lhsT = stride0(kT8[base : base + 64, hp : hp + 1,
                                   128 * kc : 128 * kc + 128])
                # matmul out must stay within one PSUM bank (<=512 f32)
                mm = [(o, 512), (512, 1024)] if o < 512 else [(o, 1024)]
                for lo, hi in mm:
                    rhs = stride0(qT8[base : base + 64, hp : hp + 1,
                                      q0 + lo : q0 + hi])
                    nc.tensor.matmul(
                        s_t[:, lo:hi], lhsT, rhs, start=True, stop=True,
                        perf_mode=PM.DoubleRow,
                    )
                w_ = 1024 - o
                if half == 1 and kc <= -1:
                    # offload early dense exps (their probs are consumed ~8
                    # periods later, hiding the DVE->Pool chain latency):
                    # DVE stages scores to SBUF, Pool computes base^s
                    s_sb = SPP.tile(
                        [128, 1024], f32, tag="ss", name=f"ss{bi}_{kc}"
                    )
                    nc.vector.tensor_copy(out=s_sb[:, 0:w_], in_=s_t[:, o:1024])
                    with nc.allow_low_precision(reason="bf16 probs"):
                        nc.gpsimd.tensor_tensor(
                            p_t[:, o:1024], ebase[:, 0:w_], s_sb[:, 0:w_],
                            OP.pow,
                        )
                else:
                    with nc.allow_low_precision(reason="bf16 probs"):
                        # DR stride-0 doubles the score; 1/16 = 0.5 * 1/8
                        nc.scalar.activation(
                            p_t[:, o:1024], s_t[:, o:1024], AF.Exp, scale=0.0625
                        )
                if half == 1 and kc >= 8:
                    # causal mask inside the diagonal 128-block. Pool for the
                    # early blocks; DVE for the last two, whose staircase
                    # would otherwise queue behind slow SWDGE y-DMAs on Pool
                    eng = nc.gpsimd
                    eng.tensor_tensor(
                        p_t[:, o : o + 128], p_t[:, o : o + 128], tri_sb[:],
                        OP.mult,
                    )
                probs[(bi, kc)] = p_t

            # per-block deferred transpose state: (pv, attq, qc)
            pend_fin = {}

            def emit_pv(bi, hp, par, half, qc, nkc):
                for kc in range(nkc):
                    ensure(("v", kc))
                h2 = 2 * hp + par
                pv = PSPV.tile([128, 512], f32, tag="pv", name=f"pv{bi}_{qc}")
                col = slice(128 * qc, 128 * qc + 128)
                for kc in range(nkc):
                    nc.tensor.matmul(
                        pv[:, 0:65],
                        probs[(bi, kc)][:, col],
                        v1[:, kc, h2, :],
                        start=(kc == 0),
                        stop=(kc == nkc - 1),
                    )
                return pv

            def emit_norm(bi, qc, pv, act=False):
                r = RP.tile([128, 1], f32, tag="r", name=f"r{bi}_{qc}")
                nc.vector.reciprocal(r[:], pv[:, 64:65])
                attq = AQP.tile([128, 64], bf16, tag="aq", name=f"aq{bi}_{qc}")
                with nc.allow_low_precision(reason="bf16 attq"):
                    if act:
                        # ACT is idle in the tail: out = in * r (per-partition)
                        nc.scalar.activation(
                            attq[:], pv[:, 0:64], AF.Copy, scale=r[:]
                        )
                    else:
                        nc.vector.tensor_scalar(
                            out=attq[:], in0=pv[:, 0:64], scalar1=r[:],
                            scalar2=None, op0=OP.mult,
                        )
                return attq

            def emit_finish(hp, par, half, qc, pv, attq, act=False):
                # transpose attq -> [64,128] into the dead pv bank, then copy
                base = 64 * par
                q0 = 1024 * half
                trout = pv[base : base + 64, 128:192].bitcast(bf16)
                nc.tensor.matmul(
                    trout, attq[:], idm_sb[:], start=True, stop=True,
                    is_transpose=True,
                )
                with nc.allow_low_precision(reason="bf16 att"):
                    dst = att[base : base + 64, hp,
                              q0 + 128 * qc : q0 + 128 * qc + 128]
                    if act:
                        nc.scalar.copy(out=dst, in_=trout)
                    else:
                        nc.vector.tensor_copy(out=dst, in_=trout)

            # ---------- output projection ----------
            yt_tiles = {}

            def emit_outproj_seg(s, nseg, late=False):
                if nseg == 0:
                    yt_tiles[s] = YP.tile([128, D], bf16, tag="y", name=f"yt{s}")
                yt = yt_tiles[s]
                ps = PSB.tile([128, 512], f32, tag="b", name=f"yps{s}_{nseg}")
                for hp in range(2):
                    nc.tensor.matmul(
                        ps[:],
                        att[:, hp, 128 * s : 128 * s + 128],
                        wo_sb[:, hp, 512 * nseg : 512 * nseg + 512],
                        start=(hp == 0),
                        stop=(hp == 1),
                    )
                sl = slice(512 * nseg, 512 * nseg + 512)
                with nc.allow_low_precision(reason="bf16 y"):
                    if s >= 12 and (s + nseg) % 2 == 0:
                        nc.scalar.copy(out=yt[:, sl], in_=ps[:])
                    else:
                        nc.vector.tensor_copy(out=yt[:, sl], in_=ps[:])
                if nseg == 1:
                    del yt_tiles[s]
                    # dual-queue issue so the sequencer cost doesn't serialize
                    # the drain; keep the slower SWDGE path off the last chunks
                    eng = nc.gpsimd if not late else nc.sync
                    eng.dma_start(y[128 * s : 128 * s + 128, :], yt[:])

            # ============================================================
            # work queue
            # ============================================================
            work = deque()  # (rows, fn, key) — filler units
            done = set()  # keys of emitted filler units
            pv_q = []  # heap of (release_period, seq, rows, fn)
            pv_seq = [0]
            period = [0]
            acct = [0.0]  # carry-over PE-row budget

            def run_unit(rows, fn, key):
                if key is not None:
                    if key in done:
                        return 0
                    done.add(key)
                fn()
                return rows

            def ensure(key):
                """Emit a queued filler unit NOW if it hasn't run yet —
                correctness guard so pacing can never reorder a consumer
                ahead of its producer."""
                if key in done:
                    return
                for i, (rows, fn, k) in enumerate(work):
                    if k == key:
                        del work[i]
                        run_unit(rows, fn, key)
                        acct[0] -= rows
                        return
                raise RuntimeError(f"missing unit {key}")

            def pv_push(release, rows_fn):
                rows, fn = rows_fn
                heapq.heappush(pv_q, (release, pv_seq[0], rows, fn))
                pv_seq[0] += 1

            def pop_rows(alloc):
                # released PV/op units first (their release period guarantees
                # deps are done, so they never stall the in-order PE stream),
                # then ready filler while the carry-over account affords it
                acct[0] = min(acct[0] + alloc, 1.5 * alloc if alloc > 0 else acct[0])
                while pv_q and pv_q[0][0] <= period[0]:
                    _, _, rows, fn = heapq.heappop(pv_q)
                    fn()
                    acct[0] -= rows
                while work and work[0][0] <= acct[0]:
                    rows, fn, key = work.popleft()
                    acct[0] -= run_unit(rows, fn, key)

            def pv_unit(bi, hp, par, half, qc, nkc):
                act = bi >= 7 and qc % 2 == 0

                def fn():
                    pv = emit_pv(bi, hp, par, half, qc, nkc)
                    attq = emit_norm(bi, qc, pv, act=act)
                    if bi in pend_fin:
                        ppv, pattq, pqc = pend_fin.pop(bi)
                        emit_finish(hp, par, half, pqc, ppv, pattq, act=act)
                        after_finish(bi, half, pqc)
                    pend_fin[bi] = (pv, attq, qc)

                return (65 * nkc + 192, fn)

            def flush_unit(bi, hp, par, half):
                act = False

                def fn():
                    if bi in pend_fin:
                        ppv, pattq, pqc = pend_fin.pop(bi)
                        emit_finish(hp, par, half, pqc, ppv, pattq, act=act)
                        after_finish(bi, half, pqc)

                return (192, fn)

            def after_finish(bi, half, qc):
                # out-projections unlock when the LAST head (block bi=3 for
                # half 0, bi=7 for half 1) lands its att chunk
                if bi == 3 and qc == 7:
                    # ration the deadline-free half-0 out-projections across
                    # blocks 4-6 so the diag stretches keep PE fed
                    for i in range(16):
                        s, nseg = i // 2, i % 2
                        pv_push(
                            period[0] + 2 + 3 * i,
                            (1024, lambda s=s, n=nseg: emit_outproj_seg(s, n)),
                        )
                elif bi == 7:
                    s = 8 + qc
                    for nseg in range(2):
                        pv_push(
                            period[0],
                            (1024, lambda s=s, n=nseg:
                             emit_outproj_seg(s, n, late=(s >= 12))),
                        )

            # ============================================================
            # emission schedule
            # ============================================================
            # --- ramp: stream x/weights, QKV chunks 0-1 ---
            nc.gpsimd.memzero(ones_sb[:])
            with nc.allow_low_precision(reason="bf16 ones"):
                nc.gpsimd.tensor_scalar_add(ones_sb[:], ones_sb[:], 1.0)
            nc.gpsimd.memset(ebase[:], float(np.exp(0.0625)))
            nc.scalar.dma_start(wq_sb[:, 0:4, :, :], wq_r[:, 0:4, :, :])
            emit_load(0, split=True)
            nc.scalar.dma_start(wq_sb[:, 4:8, :, :], wq_r[:, 4:8, :, :])
            nc.scalar.dma_start(wk_sb[:, 0:4, :, :], wk_r[:, 0:4, :, :])
            nc.scalar.dma_start(wk_sb[:, 4:8, :, :], wk_r[:, 4:8, :, :])
            nc.scalar.dma_start(wv_sb[:], wv_r[:])
            nc.gpsimd.dma_start(bq_sb[:], bqd[:].rearrange("(m p) -> p m", p=128))
            nc.gpsimd.dma_start(bv_sb[:], bvd[None, :])
            nc.scalar.dma_start(tri_sb[:], trid[:])
            nc.scalar.dma_start(idm_sb[:], idmd[:])
            # warmup matmuls: climb the PE p-state while DMAs stream
            for w in range(8):
                junk = PSB.tile([128, 512], f32, tag="b", name=f"warm{w}")
                nc.tensor.matmul(
                    junk[:], ones_sb[0:1, :128], ones_sb[:], start=True, stop=True
                )
            # minimal pre-B0 projections: q(0,0) and k(0,0), then the first
            # two kc's lo-half scores fire before q(1,0) is even projected
            emit_q(0, 0, act_copy=True)
            emit_k(0, 0, act_copy=True)
            emit_load(1)
            done.update({("q", 0, 0, 0), ("q", 0, 0, 256),
                         ("k", 0, 0, 0), ("k", 0, 0, 256)})
            hp0, par0, _ = None, None, None
            emit_scores(0, 0, 0, 0, 0, split=True)
            emit_scores(0, 0, 0, 0, 1, split=True)
            emit_q(1, 0, act_copy=True)
            done.update({("q", 1, 0, 0), ("q", 1, 0, 256)})
            nc.sync.dma_start(v1[:, :, :, 64:65], onesd[0:64].partition_broadcast(128))
            emit_load(2)
            emit_load(3)
            nc.scalar.dma_start(wo_sb[:], wo_r[:])

            def push_qk(which, n4, m):
                for toff in (0, 256):
                    work.append(
                        (1536,
                         lambda n4=n4, m=m, toff=toff:
                         emit_qk_part(which, n4, m, toff, 256),
                         (which, n4, m, toff))
                    )

            # --- remaining QKV queued as filler, in deadline order ---
            # v0-7 + k(1,0): B0; m=1 units: B2 kc0; q n4 2-3: B4; v8-15: B5
            def push_v(s):
                work.append((1792, lambda s=s: emit_v(s), ("v", s)))

            push_v(0)
            push_v(1)
            push_qk("k", 1, 0)
            for s in range(2, 8):
                push_v(s)
            push_qk("q", 0, 1)
            push_qk("k", 0, 1)
            push_qk("q", 1, 1)
            push_qk("k", 1, 1)
            push_qk("q", 2, 0)
            push_qk("q", 3, 0)
            push_qk("q", 2, 1)
            push_qk("q", 3, 1)
            push_qk("k", 2, 0)
            push_qk("k", 3, 0)
            for s in range(8, 12):
                push_v(s)
            push_qk("k", 2, 1)
            push_qk("k", 3, 1)
            for s in range(12, 16):
                push_v(s)

            # --- main blocks ---
            blocks = [(hp, par, half) for half in (0, 1) for hp, par in HEADS]
            for bi, (hp, par, half) in enumerate(blocks):
                nkcs = 8 if half == 0 else 16
                for kc in range(nkcs):
                    o = 0 if (half == 0 or kc < 8) else 128 * (kc - 8)
                    if bi == 0 and kc < 2:
                        # lo halves already emitted in the ramp
                        emit_scores_piece(bi, hp, par, half, kc, 512, 1024)
                    else:
                        emit_scores(bi, hp, par, half, kc)
                    if half == 0:
                        if kc == 7:
                            # spread the 8 units across the next block's
                            # periods to avoid a block-boundary burst
                            for qc in range(8):
                                pv_push(
                                    period[0] + 1 + (3 * qc) // 4,
                                    pv_unit(bi, hp, par, half, qc, 8),
                                )
                            pv_push(period[0] + 7, flush_unit(bi, hp, par, half))
                    else:
                        if kc >= 8:
                            qc = kc - 8
                            pv_push(
                                period[0] + 1,
                                pv_unit(bi, hp, par, half, qc, kc + 1),
                            )
                            if kc == 15:
                                pv_push(
                                    period[0] + 1, flush_unit(bi, hp, par, half)
                                )
                    # pace the queue at ~1.05x the exp cadence so ACT (not PE)
                    # absorbs scheduling jitter; scores rows count against it
                    w_ = 1024 - o
                    exp_rows = (0.833 * w_ + 185.0) / 0.4167
                    alloc = 0.93 * exp_rows - w_ // 2
                    if bi >= 6:
                        alloc = max(alloc, 3400.0)
                    pop_rows(alloc)
                    period[0] += 1

            # --- drain ---
            period[0] += 1000
            while pv_q or work:
                while pv_q:
                    _, _, rows, fn = heapq.heappop(pv_q)
                    fn()
                while work:
                    rows, fn = work.popleft()
                    fn()

    nc.compile()
    return nc


_NC = None


def _get_nc():
    global _NC
    if _NC is None:
        _NC = build_nc()
    return _NC


def make_in_maps(x, Wq, bq, Wk, bk, Wv, bv, Wo):
    _get_nc()
    bf = ml_dtypes.bfloat16
    e4 = ml_dtypes.float8_e4m3fn
    x = np.asarray(x, np.float32)
    kk = np.arange(128)[:, None]
    qp = np.arange(128)[None, :]
    tri = (kk <= qp).astype(bf)
    idm = np.eye(128, dtype=np.float32).astype(bf)
    ones = np.ones(512, bf)

    def hilo(a):
        h = a.astype(e4)
        l = (a - h.astype(np.float32)).astype(e4)
        return h, l

    def w8(W, sl):
        # [D, 2, HG]: hi/lo of 16*W
        h, l = hilo(np.asarray(W, np.float32)[:, sl] * 16.0)
        return np.ascontiguousarray(np.stack([h, l], axis=1))

    in_maps = []
    for core in range(8):
        b, g = core // 4, core % 4
        sl = slice(HG * g, HG * (g + 1))
        xh, xl = hilo(x[b].T)
        in_maps.append(
            {
                "xth": np.ascontiguousarray(xh),
                "xtl": np.ascontiguousarray(xl),
                "wq8": w8(Wq, sl),
                "wk8": w8(Wk, sl),
                "wv8": w8(Wv, sl),
                "bq": np.ascontiguousarray(np.asarray(bq, np.float32)[sl]),
                "bv": np.ascontiguousarray(
                    (np.asarray(bv, np.float32)[sl] * 16.0).astype(bf)
                ),
                "wo": np.ascontiguousarray(np.asarray(Wo, np.float32)[sl, :].astype(bf)),
                "tri": tri,
                "idm": idm,
                "ones": ones,
            }
        )
    return in_maps


def kernel(x, Wq, bq, Wk, bk, Wv, bv, Wo, _trace=False, _trace_kwargs=None):
    nc = _get_nc()
    in_maps = make_in_maps(x, Wq, bq, Wk, bk, Wv, bv, Wo)
    res = run_bass_kernel_spmd(
        nc, in_maps, list(range(8)), trace=_trace, **(_trace_kwargs or {})
    )
    out = np.zeros((2, N, D), np.float64)
    for core in range(8):
        out[core // 4] += np.asarray(res.results[core]["y"], np.float64)
    yf = out.astype(np.float32)
    if _trace:
        return yf, res
    return yf


# revision 5
# speedup vs baseline: 1.0219x; 1.0022x over previous
"""Sparse (half-causal) multi-head attention on 8 Trainium2 NeuronCores, v3.

Problem: x[2,2048,1024] -> QKV proj (16 heads, dk=dv=64) -> scores with
half-causal mask (rows <1024 attend cols <1024 dense; rows >=1024 causal)
-> softmax -> out proj.

Sharding: 8 cores = 2 batches x 4 head-groups (4 heads each).  Each core
computes its batch's full QKV for its 4 heads (column-sharded W), attention
for those heads, and a partial output projection (row-sharded Wo).  Host
sums the 4 partials per batch.

v3 changes vs v2 (152.3us -> 123.3us):
 - scores in fp8 e4m3 via DoubleRow perf mode with a stride-0 k-tile slot:
   both DR slots read the same data, computing 2x(k.q) at 0.5 cyc/row; the
   doubling folds into the exp scale (0.0625).  Halves scores PE time.
 - QKV projections as 3-term hi/lo fp8 DoubleRow (xh@Wh + xl@Wh + xh@Wl,
   weights x16 against e4m3 subnormals, 1/16 folded into the psum copy):
   0.75x the bf16 matmul rows at ~bf16 accuracy (rel err 1.5e-2 vs 2e-2
   gate, measured on HW).
 - PV flipped: out[q,v] = probs[k,q].T @ v[k,v] per 128-q chunk, cost 65
   rows/chunk instead of 128 (the 65-wide v+denominator free dim is the
   cheap side).  Denominator becomes a per-partition scalar: recip +
   tensor_scalar multiply, killing v2's dscr broadcast-DMA machinery.
 - att transposed back via PE transpose (identity moving operand) with the
   bf16 output bitcast into the just-consumed pv psum bank (no extra bank).
 - no y2 partial: the causal staircase finishes head (1,1) q-chunk j at
   kc 8+j, so the full out-projection (both head pairs) rides the diagonal.
 - work-queue scheduler paced to the exp cadence: filler (QKV chunks, out-
   projections) pops between scores/exp emissions under a carry-over row
   budget; PV/norm units release one period after their probs so the
   in-order PE stream never stalls on exp; deadline `ensure` pulls keep
   correctness independent of pacing; half-0 out-projections are rationed
   across the half-1 dense blocks to feed PE through the ACT-bound stretch.

Engine budget (TimelineSim): ACT 98.7us (exp-bound), PE 88.0us,
DVE 55us, Pool 29us -> 123.3us total (drain copies alternate ACT/DVE;
the first two kc's lo-half scores fire in the ramp).
"""

import copy as _copy
import heapq
import sys
from collections import deque

if "/opt/trn_rl_repo" not in sys.path:
    sys.path.insert(0, "/opt/trn_rl_repo")

import ml_dtypes
import numpy as np

import concourse.bass as bass  # noqa: F401 (import registers engines)
import concourse.mybir as mybir
import concourse.tile as tile
from concourse import bacc
from concourse.bass_utils import run_bass_kernel_spmd

f32 = mybir.dt.float32
bf16 = mybir.dt.bfloat16
fp8 = mybir.dt.float8e4
AF = mybir.ActivationFunctionType
OP = mybir.AluOpType
PM = mybir.MatmulPerfMode

D = 1024  # d_model
N = 2048  # n_ctx
HG = 256  # head-group width per core (4 heads x 64)

HEADS = [(0, 0), (0, 1), (1, 0), (1, 1)]  # (hp, par)


def stride0(ap):
    """Turn the first singleton non-partition dim into a stride-0 2-count
    dim (the DoubleRow k-tile slot reading the same data twice)."""
    ap2 = _copy.copy(ap)
    lst = ap2.ap
    for i in range(1, len(lst)):
        if lst[i][1] == 1:
            lst[i] = [0, 2]
            return ap2
    raise RuntimeError(f"no singleton dim in {lst}")


def build_nc():
    nc = bacc.Bacc("TRN2", target_bir_lowering=False, debug=False)

    # x and the QKV weights ship as fp8 hi/lo pairs (weights pre-scaled x16
    # on the host; the 1/16 folds into the psum->sbuf copy)
    xth = nc.declare_dram_parameter("xth", [D, N], fp8, isOutput=False)
    xtl = nc.declare_dram_parameter("xtl", [D, N], fp8, isOutput=False)
    wq8 = nc.declare_dram_parameter("wq8", [D, 2, HG], fp8, isOutput=False)
    wk8 = nc.declare_dram_parameter("wk8", [D, 2, HG], fp8, isOutput=False)
    wv8 = nc.declare_dram_parameter("wv8", [D, 2, HG], fp8, isOutput=False)
    bqd = nc.declare_dram_parameter("bq", [HG], f32, isOutput=False)
    bvd = nc.declare_dram_parameter("bv", [HG], bf16, isOutput=False)
    wo = nc.declare_dram_parameter("wo", [HG, D], bf16, isOutput=False)
    trid = nc.declare_dram_parameter("tri", [128, 128], bf16, isOutput=False)
    idmd = nc.declare_dram_parameter("idm", [128, 128], bf16, isOutput=False)
    onesd = nc.declare_dram_parameter("ones", [512], bf16, isOutput=False)
    y = nc.declare_dram_parameter("y", [N, D], bf16, isOutput=True)

    xth_r = xth[:].rearrange("(c p) n -> p c n", p=128)
    xtl_r = xtl[:].rearrange("(c p) n -> p c n", p=128)
    wq_r = wq8[:].rearrange("(c p) hl m -> p c hl m", p=128)
    wk_r = wk8[:].rearrange("(c p) hl m -> p c hl m", p=128)
    wv_r = wv8[:].rearrange("(c p) hl m -> p c hl m", p=128)
    wo_r = wo[:].rearrange("(c p) n -> p c n", p=128)

    with tile.TileContext(nc) as tc:
        with (
            tc.tile_pool(name="persist", bufs=1) as P1,
            tc.tile_pool(name="xtp", bufs=8) as XTP,
            tc.tile_pool(name="ppool", bufs=40) as PP,
            tc.tile_pool(name="aqp", bufs=4) as AQP,
            tc.tile_pool(name="spp", bufs=2) as SPP,
            tc.tile_pool(name="rp", bufs=4) as RP,
            tc.tile_pool(name="yp", bufs=4) as YP,
            tc.tile_pool(name="ps_s", bufs=2, space="PSUM") as PSS,
            tc.tile_pool(name="ps_pv", bufs=2, space="PSUM") as PSPV,
            tc.tile_pool(name="ps_b", bufs=2, space="PSUM") as PSB,
        ):
            # ---------- persistent tiles ----------
            # [part, c-chunk, hi/lo, cols]: the (c, c+1) pairing for DR slots
            # lives in the c dim; hi/lo selects the split
            wq_sb = P1.tile([128, 8, 2, HG], fp8, tag="wq")
            wk_sb = P1.tile([128, 8, 2, HG], fp8, tag="wk")
            wv_sb = P1.tile([128, 8, 2, HG], fp8, tag="wv")
            wo_sb = P1.tile([128, 2, D], bf16, tag="wo")
            bq_sb = P1.tile([128, 2], f32, tag="bq")
            bv_sb = P1.tile([1, HG], bf16, tag="bv")
            ones_sb = P1.tile([1, 512], bf16, tag="ones")
            tri_sb = P1.tile([128, 128], bf16, tag="tri")
            idm_sb = P1.tile([128, 128], bf16, tag="idm")

            ebase = P1.tile([128, 1024], f32, tag="ebase")
            qT8 = P1.tile([128, 2, N], fp8, tag="qT8")
            kT8 = P1.tile([128, 2, N], fp8, tag="kT8")
            v1 = P1.tile([128, 16, 4, 65], bf16, tag="v1")
            att = P1.tile([128, 2, N], bf16, tag="att")

            # ---------- QKV emitters ----------
            xt_tiles = {}

            def emit_load(n4, split=False):
                ns = slice(512 * n4, 512 * n4 + 512)
                a = XTP.tile([128, 8, 512], fp8, tag="xt", name=f"xth{n4}")
                b_ = XTP.tile([128, 8, 512], fp8, tag="xt", name=f"xtl{n4}")
                for h in range(2):
                    cs = slice(4 * h, 4 * h + 4)
                    nc.sync.dma_start(a[:, cs, :], xth_r[:, cs, ns])
                eng = nc.gpsimd if split else nc.sync
                for h in range(2):
                    cs = slice(4 * h, 4 * h + 4)
                    eng.dma_start(b_[:, cs, :], xtl_r[:, cs, ns])
                xt_tiles[n4] = (a, b_)

            def emit_qk_part(which, n4, m, toff, twid, act_copy=False):
                """Q/K projection for a token sub-range via 3-term hi/lo fp8
                DoubleRow (xh@Wh + xl@Wh + xh@Wl); weights are x16 so the
                psum->sbuf copy scales by 1/16 (and adds bq for Q)."""
                ns = slice(512 * n4 + toff, 512 * n4 + toff + twid)
                xh, xl = xt_tiles[n4]
                msl = slice(128 * m, 128 * m + 128)
                w_sb = wq_sb if which == "q" else wk_sb
                ps = PSB.tile(
                    [128, 512], f32, tag="b", name=f"{which}ps{n4}{m}{toff}"
                )
                terms = [(xh, 0), (xl, 0), (xh, 1)]
                for ti, (xs, hl) in enumerate(terms):
                    for j in range(4):
                        nc.tensor.matmul(
                            ps[:, 0:twid],
                            w_sb[:, 2 * j : 2 * j + 2, hl, msl],
                            xs[:, 2 * j : 2 * j + 2, toff : toff + twid],
                            start=(ti == 0 and j == 0),
                            stop=(ti == 2 and j == 3),
                            perf_mode=PM.DoubleRow,
                        )
                if which == "q":
                    with nc.allow_low_precision(reason="fp8 qT"):
                        if act_copy:
                            nc.scalar.activation(
                                qT8[:, m, ns], ps[:, 0:twid], AF.Identity,
                                bias=bq_sb[:, m : m + 1], scale=1.0 / 16.0,
                            )
                        else:
                            nc.vector.tensor_scalar(
                                out=qT8[:, m, ns], in0=ps[:, 0:twid],
                                scalar1=1.0 / 16.0,
                                scalar2=bq_sb[:, m : m + 1],
                                op0=OP.mult, op1=OP.add,
                            )
                else:
                    with nc.allow_low_precision(reason="fp8 kT"):
                        if act_copy:
                            nc.scalar.mul(kT8[:, m, ns], ps[:, 0:twid], 1.0 / 16.0)
                        else:
                            nc.vector.tensor_scalar(
                                out=kT8[:, m, ns], in0=ps[:, 0:twid],
                                scalar1=1.0 / 16.0, scalar2=None, op0=OP.mult,
                            )

            def emit_q(n4, m, act_copy=False):
                emit_qk_part("q", n4, m, 0, 512, act_copy=act_copy)

            def emit_k(n4, m, act_copy=False):
                emit_qk_part("k", n4, m, 0, 512, act_copy=act_copy)

            def emit_v(s, act_copy=False):
                n4 = s // 4
                xh, xl = xt_tiles[n4]
                so = 128 * (s - 4 * n4)
                ps = PSB.tile([128, 256], f32, tag="b", name=f"vps{s}")
                terms = [(xh, 0), (xl, 0), (xh, 1)]
                for ti, (xs, hl) in enumerate(terms):
                    for j in range(4):
                        nc.tensor.matmul(
                            ps[:],
                            xs[:, 2 * j : 2 * j + 2, so : so + 128],
                            wv_sb[:, 2 * j : 2 * j + 2, hl, :],
                            start=(ti == 0 and j == 0),
                            stop=False,
                            perf_mode=PM.DoubleRow,
                        )
                # bias (x16 on host, like the weights)
                nc.tensor.matmul(
                    ps[:], ones_sb[:, :128], bv_sb[:], start=False, stop=True
                )
                with nc.allow_low_precision(reason="bf16 v1"):
                    src = ps[:].rearrange("p (h d) -> p h d", h=4)
                    if act_copy:
                        nc.scalar.mul(v1[:, s, :, 0:64], src, 1.0 / 16.0)
                    else:
                        nc.vector.tensor_scalar(
                            out=v1[:, s, :, 0:64], in0=src,
                            scalar1=1.0 / 16.0, scalar2=None, op0=OP.mult,
                        )

            # ---------- attention ----------
            probs = {}  # (bi, kc) -> p_t

            sc_tiles = {}

            def emit_scores_piece(bi, hp, par, half, kc, lo, hi):
                """One <=512-wide scores matmul + its exp piece."""
                q0 = 1024 * half
                base = 64 * par
                ensure(("q", 2 * half + lo // 512, hp, 0))
                ensure(("q", 2 * half + lo // 512, hp, 256))
                if hi - lo > 512 - 256:  # piece spans into the next q-256
                    ensure(("q", 2 * half + (hi - 1) // 512, hp, 0))
                    ensure(("q", 2 * half + (hi - 1) // 512, hp, 256))
                ensure(("k", kc // 4, hp, 256 * ((kc % 4) // 2)))
                s_t, p_t = sc_tiles[(bi, kc)]
                lhsT = stride0(kT8[base : base + 64, hp : hp + 1,
                                   128 * kc : 128 * kc + 128])
                rhs = stride0(qT8[base : base + 64, hp : hp + 1,
                                  q0 + lo : q0 + hi])
                nc.tensor.matmul(
                    s_t[:, lo:hi], lhsT, rhs, start=True, stop=True,
                    perf_mode=PM.DoubleRow,
                )
                with nc.allow_low_precision(reason="bf16 probs"):
                    # DR stride-0 doubles the score; 1/16 = 0.5 * 1/8
                    nc.scalar.activation(
                        p_t[:, lo:hi], s_t[:, lo:hi], AF.Exp, scale=0.0625
                    )

            def emit_scores(bi, hp, par, half, kc, split=False):
                q0 = 1024 * half
                base = 64 * par
                o = 0 if (half == 0 or kc < 8) else 128 * (kc - 8)
                s_t = PSS.tile([128, 1024], f32, tag="s", name=f"s{bi}_{kc}")
                p_t = PP.tile([128, 1024], bf16, tag="p", name=f"p{bi}_{kc}")
                sc_tiles[(bi, kc)] = (s_t, p_t)
                probs[(bi, kc)] = p_t
                if split:
                    # B0 ramp compression: emit only the lo half now; the hi
                    # half (needing q(1,0)) is emitted via emit_scores_piece
                    emit_scores_piece(bi, hp, par, half, kc, 0, 512)
                    return
                # correctness guards: prerequisite projections must be emitted
                for n4 in (2 * half, 2 * half + 1):
                    for toff in (0, 256):
                        ensure(("q", n4, hp, toff))
                ensure(("k", kc // 4, hp, 256 * ((kc % 4) // 2)))
                lhsT = stride0(kT8[base : base + 64, hp : hp + 1,
                                   128 * kc : 128 * kc + 128])
                # matmul out must stay within one PSUM bank (<=512 f32)
                mm = [(o, 512), (512, 1024)] if o < 512 else [(o, 1024)]
                for lo, hi in mm:
                    rhs = stride0(qT8[base : base + 64, hp : hp + 1,
                                      q0 + lo : q0 + hi])
                    nc.tensor.matmul(
                        s_t[:, lo:hi], lhsT, rhs, start=True, stop=True,
                        perf_mode=PM.DoubleRow,
                    )
                w_ = 1024 - o
                if half == 1 and kc <= -1:
                    # offload early dense exps (their probs are consumed ~8
                    # periods later, hiding the DVE->Pool chain latency):
                    # DVE stages scores to SBUF, Pool computes base^s
                    s_sb = SPP.tile(
                        [128, 1024], f32, tag="ss", name=f"ss{bi}_{kc}"
                    )
                    nc.vector.tensor_copy(out=s_sb[:, 0:w_], in_=s_t[:, o:1024])
                    with nc.allow_low_precision(reason="bf16 probs"):
                        nc.gpsimd.tensor_tensor(
                            p_t[:, o:1024], ebase[:, 0:w_], s_sb[:, 0:w_],
                            OP.pow,
                        )
                else:
                    with nc.allow_low_precision(reason="bf16 probs"):
                        # DR stride-0 doubles the score; 1/16 = 0.5 * 1/8
                        nc.scalar.activation(
                            p_t[:, o:1024], s_t[:, o:1024], AF.Exp, scale=0.0625
                        )
                if half == 1 and kc >= 8:
                    # causal mask inside the diagonal 128-block. Pool for the
                    # early blocks; DVE for the last two, whose staircase
                    # would otherwise queue behind slow SWDGE y-DMAs on Pool
                    eng = nc.gpsimd
                    eng.tensor_tensor(
                        p_t[:, o : o + 128], p_t[:, o : o + 128], tri_sb[:],
                        OP.mult,
                    )
                probs[(bi, kc)] = p_t

            # per-block deferred transpose state: (pv, attq, qc)
            pend_fin = {}

            def emit_pv(bi, hp, par, half, qc, nkc):
                for kc in range(nkc):
                    ensure(("v", kc))
                h2 = 2 * hp + par
                pv = PSPV.tile([128, 512], f32, tag="pv", name=f"pv{bi}_{qc}")
                col = slice(128 * qc, 128 * qc + 128)
                for kc in range(nkc):
                    nc.tensor.matmul(
                        pv[:, 0:65],
                        probs[(bi, kc)][:, col],
                        v1[:, kc, h2, :],
                        start=(kc == 0),
                        stop=(kc == nkc - 1),
                    )
                return pv

            def emit_norm(bi, qc, pv, act=False):
                r = RP.tile([128, 1], f32, tag="r", name=f"r{bi}_{qc}")
                nc.vector.reciprocal(r[:], pv[:, 64:65])
                attq = AQP.tile([128, 64], bf16, tag="aq", name=f"aq{bi}_{qc}")
                with nc.allow_low_precision(reason="bf16 attq"):
                    if act:
                        # ACT is idle in the tail: out = in * r (per-partition)
                        nc.scalar.activation(
                            attq[:], pv[:, 0:64], AF.Copy, scale=r[:]
                        )
                    else:
                        nc.vector.tensor_scalar(
                            out=attq[:], in0=pv[:, 0:64], scalar1=r[:],
                            scalar2=None, op0=OP.mult,
                        )
                return attq

            def emit_finish(hp, par, half, qc, pv, attq, act=False):
                # transpose attq -> [64,128] into the dead pv bank, then copy
                base = 64 * par
                q0 = 1024 * half
                trout = pv[base : base + 64, 128:192].bitcast(bf16)
                nc.tensor.matmul(
                    trout, attq[:], idm_sb[:], start=True, stop=True,
                    is_transpose=True,
                )
                with nc.allow_low_precision(reason="bf16 att"):
                    dst = att[base : base + 64, hp,
                              q0 + 128 * qc : q0 + 128 * qc + 128]
                    if act:
                        nc.scalar.copy(out=dst, in_=trout)
                    else:
                        nc.vector.tensor_copy(out=dst, in_=trout)

            # ---------- output projection ----------
            yt_tiles = {}

            def emit_outproj_seg(s, nseg, late=False):
                if nseg == 0:
                    yt_tiles[s] = YP.tile([128, D], bf16, tag="y", name=f"yt{s}")
                yt = yt_tiles[s]
                ps = PSB.tile([128, 512], f32, tag="b", name=f"yps{s}_{nseg}")
                for hp in range(2):
                    nc.tensor.matmul(
                        ps[:],
                        att[:, hp, 128 * s : 128 * s + 128],
                        wo_sb[:, hp, 512 * nseg : 512 * nseg + 512],
                        start=(hp == 0),
                        stop=(hp == 1),
                    )
                sl = slice(512 * nseg, 512 * nseg + 512)
                with nc.allow_low_precision(reason="bf16 y"):
                    if s >= 12 and (s + nseg) % 2 == 0:
                        nc.scalar.copy(out=yt[:, sl], in_=ps[:])
                    else:
                        nc.vector.tensor_copy(out=yt[:, sl], in_=ps[:])
                if nseg == 1:
                    del yt_tiles[s]
                    # dual-queue issue so the sequencer cost doesn't serialize
                    # the drain; keep the slower SWDGE path off the last chunks
                    eng = nc.gpsimd if not late else nc.sync
                    eng.dma_start(y[128 * s : 128 * s + 128, :], yt[:])

            # ============================================================
            # work queue
            # ============================================================
            work = deque()  # (rows, fn, key) — filler units
            done = set()  # keys of emitted filler units
            pv_q = []  # heap of (release_period, seq, rows, fn)
            pv_seq = [0]
            period = [0]
            acct = [0.0]  # carry-over PE-row budget

            def run_unit(rows, fn, key):
                if key is not None:
                    if key in done:
                        return 0
                    done.add(key)
                fn()
                return rows

            def ensure(key):
                """Emit a queued filler unit NOW if it hasn't run yet —
                correctness guard so pacing can never reorder a consumer
                ahead of its producer."""
                if key in done:
                    return
                for i, (rows, fn, k) in enumerate(work):
                    if k == key:
                        del work[i]
                        run_unit(rows, fn, key)
                        acct[0] -= rows
                        return
                raise RuntimeError(f"missing unit {key}")

            def pv_push(release, rows_fn):
                rows, fn = rows_fn
                heapq.heappush(pv_q, (release, pv_seq[0], rows, fn))
                pv_seq[0] += 1

            def pop_rows(alloc):
                # released PV/op units first (their release period guarantees
                # deps are done, so they never stall the in-order PE stream),
                # then ready filler while the carry-over account affords it
                acct[0] = min(acct[0] + alloc, 1.5 * alloc if alloc > 0 else acct[0])
                while pv_q and pv_q[0][0] <= period[0]:
                    _, _, rows, fn = heapq.heappop(pv_q)
                    fn()
                    acct[0] -= rows
                while work and work[0][0] <= acct[0]:
                    rows, fn, key = work.popleft()
                    acct[0] -= run_unit(rows, fn, key)

            def pv_unit(bi, hp, par, half, qc, nkc):
                act = bi >= 7 and qc % 2 == 0

                def fn():
                    pv = emit_pv(bi, hp, par, half, qc, nkc)
                    attq = emit_norm(bi, qc, pv, act=act)
                    if bi in pend_fin:
                        ppv, pattq, pqc = pend_fin.pop(bi)
                        emit_finish(hp, par, half, pqc, ppv, pattq, act=act)
                        after_finish(bi, half, pqc)
                    pend_fin[bi] = (pv, attq, qc)

                return (65 * nkc + 192, fn)

            def flush_unit(bi, hp, par, half):
                act = False

                def fn():
                    if bi in pend_fin:
                        ppv, pattq, pqc = pend_fin.pop(bi)
                        emit_finish(hp, par, half, pqc, ppv, pattq, act=act)
                        after_finish(bi, half, pqc)

                return (192, fn)

            def after_finish(bi, half, qc):
                # out-projections unlock when the LAST head (block bi=3 for
                # half 0, bi=7 for half 1) lands its att chunk
                if bi == 3 and qc == 7:
                    # ration the deadline-free half-0 out-projections across
                    # blocks 4-6 so the diag stretches keep PE fed
                    for i in range(16):
                        s, nseg = i // 2, i % 2
                        pv_push(
                            period[0] + 2 + 3 * i,
                            (1024, lambda s=s, n=nseg: emit_outproj_seg(s, n)),
                        )
                elif bi == 7:
                    s = 8 + qc
                    for nseg in range(2):
                        pv_push(
                            period[0],
                            (1024, lambda s=s, n=nseg:
                             emit_outproj_seg(s, n, late=(s >= 12))),
                        )

            # ============================================================
            # emission schedule
            # ============================================================
            # --- ramp: stream x/weights, QKV chunks 0-1 ---
            nc.gpsimd.memzero(ones_sb[:])
            with nc.allow_low_precision(reason="bf16 ones"):
                nc.gpsimd.tensor_scalar_add(ones_sb[:], ones_sb[:], 1.0)
            nc.gpsimd.memset(ebase[:], float(np.exp(0.0625)))
            nc.scalar.dma_start(wq_sb[:, 0:4, :, :], wq_r[:, 0:4, :, :])
            emit_load(0, split=True)
            nc.scalar.dma_start(wq_sb[:, 4:8, :, :], wq_r[:, 4:8, :, :])
            nc.scalar.dma_start(wk_sb[:, 0:4, :, :], wk_r[:, 0:4, :, :])
            nc.scalar.dma_start(wk_sb[:, 4:8, :, :], wk_r[:, 4:8, :, :])
            nc.scalar.dma_start(wv_sb[:], wv_r[:])
            nc.gpsimd.dma_start(bq_sb[:], bqd[:].rearrange("(m p) -> p m", p=128))
            nc.gpsimd.dma_start(bv_sb[:], bvd[None, :])
            nc.scalar.dma_start(tri_sb[:], trid[:])
            nc.scalar.dma_start(idm_sb[:], idmd[:])
            # warmup matmuls: climb the PE p-state while DMAs stream
            for w in range(8):
                junk = PSB.tile([128, 512], f32, tag="b", name=f"warm{w}")
                nc.tensor.matmul(
                    junk[:], ones_sb[0:1, :128], ones_sb[:], start=True, stop=True
                )
            # minimal pre-B0 projections: q(0,0) and k(0,0), then the first
            # two kc's lo-half scores fire before q(1,0) is even projected
            emit_q(0, 0, act_copy=True)
            emit_k(0, 0, act_copy=True)
            emit_load(1)
            done.update({("q", 0, 0, 0), ("q", 0, 0, 256),
                         ("k", 0, 0, 0), ("k", 0, 0, 256)})
            hp0, par0, _ = None, None, None
            emit_scores(0, 0, 0, 0, 0, split=True)
            emit_scores(0, 0, 0, 0, 1, split=True)
            emit_q(1, 0, act_copy=True)
            done.update({("q", 1, 0, 0), ("q", 1, 0, 256)})
            nc.sync.dma_start(v1[:, :, :, 64:65], onesd[0:64].partition_broadcast(128))
            emit_load(2)
            emit_load(3)
            nc.scalar.dma_start(wo_sb[:], wo_r[:])

            def push_qk(which, n4, m):
                for toff in (0, 256):
                    work.append(
                        (1536,
                         lambda n4=n4, m=m, toff=toff:
                         emit_qk_part(which, n4, m, toff, 256),
                         (which, n4, m, toff))
                    )

            # --- remaining QKV queued as filler, in deadline order ---
            # v0-7 + k(1,0): B0; m=1 units: B2 kc0; q n4 2-3: B4; v8-15: B5
            def push_v(s):
                work.append((1792, lambda s=s: emit_v(s), ("v", s)))

            push_v(0)
            push_v(1)
            push_qk("k", 1, 0)
            for s in range(2, 8):
                push_v(s)
            push_qk("q", 0, 1)
            push_qk("k", 0, 1)
            push_qk("q", 1, 1)
            push_qk("k", 1, 1)
            push_qk("q", 2, 0)
            push_qk("q", 3, 0)
            push_qk("q", 2, 1)
            push_qk("q", 3, 1)
            push_qk("k", 2, 0)
            push_qk("k", 3, 0)
            for s in range(8, 12):
                push_v(s)
            push_qk("k", 2, 1)
            push_qk("k", 3, 1)
            for s in range(12, 16):
                push_v(s)

            # --- main blocks ---
            blocks = [(hp, par, half) for half in (0, 1) for hp, par in HEADS]
            for bi, (hp, par, half) in enumerate(blocks):
                nkcs = 8 if half == 0 else 16
                for kc in range(nkcs):
                    o = 0 if (half == 0 or kc < 8) else 128 * (kc - 8)
                    if bi == 0 and kc < 2:
                        # lo halves already emitted in the ramp
                        emit_scores_piece(bi, hp, par, half, kc, 512, 1024)
                    else:
                        emit_scores(bi, hp, par, half, kc)
                    if half == 0:
                        if kc == 7:
                            # spread the 8 units across the next block's
                            # periods to avoid a block-boundary burst
                            for qc in range(8):
                                pv_push(
                                    period[0] + 1 + (3 * qc) // 4,
                                    pv_unit(bi, hp, par, half, qc, 8),
                                )
                            pv_push(period[0] + 7, flush_unit(bi, hp, par, half))
                    else:
                        if kc >= 8:
                            qc = kc - 8
                            pv_push(
                                period[0] + 1,
                                pv_unit(bi, hp, par, half, qc, kc + 1),
                            )
                            if kc == 15:
                                pv_push(
                                    period[0] + 1, flush_unit(bi, hp, par, half)
                                )
                    # pace the queue at ~1.05x the exp cadence so ACT (not PE)
                    # absorbs scheduling jitter; scores rows count against it
                    w_ = 1024 - o
                    exp_rows = (0.833 * w_ + 185.0) / 0.4167
                    alloc = 0.95 * exp_rows - w_ // 2
                    if bi >= 6:
                        alloc = max(alloc, 3400.0)
                    pop_rows(alloc)
                    period[0] += 1

            # --- drain ---
            period[0] += 1000
            while pv_q or work:
                while pv_q:
                    _, _, rows, fn = heapq.heappop(pv_q)
                    fn()
                while work:
                    rows, fn = work.popleft()
                    fn()

    nc.compile()
    return nc


_NC = None


def _get_nc():
    global _NC
    if _NC is None:
        _NC = build_nc()
    return _NC


def make_in_maps(x, Wq, bq, Wk, bk, Wv, bv, Wo):
    _get_nc()
    bf = ml_dtypes.bfloat16
    e4 = ml_dtypes.float8_e4m3fn
    x = np.asarray(x, np.float32)
    kk = np.arange(128)[:, None]
    qp = np.arange(128)[None, :]
    tri = (kk <= qp).astype(bf)
    idm = np.eye(128, dtype=np.float32).astype(bf)
    ones = np.ones(512, bf)

    def hilo(a):
        h = a.astype(e4)
        l = (a - h.astype(np.float32)).astype(e4)
        return h, l

    def w8(W, sl):
        # [D, 2, HG]: hi/lo of 16*W
        h, l = hilo(np.asarray(W, np.float32)[:, sl] * 16.0)
        return np.ascontiguousarray(np.stack([h, l], axis=1))

    in_maps = []
    for core in range(8):
        b, g = core // 4, core % 4
        sl = slice(HG * g, HG * (g + 1))
        xh, xl = hilo(x[b].T)
        in_maps.append(
            {
                "xth": np.ascontiguousarray(xh),
                "xtl": np.ascontiguousarray(xl),
                "wq8": w8(Wq, sl),
                "wk8": w8(Wk, sl),
                "wv8": w8(Wv, sl),
                "bq": np.ascontiguousarray(np.asarray(bq, np.float32)[sl]),
                "bv": np.ascontiguousarray(
                    (np.asarray(bv, np.float32)[sl] * 16.0).astype(bf)
                ),
                "wo": np.ascontiguousarray(np.asarray(Wo, np.float32)[sl, :].astype(bf)),
                "tri": tri,
                "idm": idm,
                "ones": ones,
            }
        )
    return in_maps


def kernel(x, Wq, bq, Wk, bk, Wv, bv, Wo, _trace=False, _trace_kwargs=None):
    nc = _get_nc()
    in_maps = make_in_maps(x, Wq, bq, Wk, bk, Wv, bv, Wo)
    res = run_bass_kernel_spmd(
        nc, in_maps, list(range(8)), trace=_trace, **(_trace_kwargs or {})
    )
    out = np.zeros((2, N, D), np.float64)
    for core in range(8):
        out[core // 4] += np.asarray(res.results[core]["y"], np.float64)
    yf = out.astype(np.float32)
    if _trace:
        return yf, res
    return yf


# revision 6
# speedup vs baseline: 1.0251x; 1.0031x over previous
"""Sparse (half-causal) multi-head attention on 8 Trainium2 NeuronCores, v3.

Problem: x[2,2048,1024] -> QKV proj (16 heads, dk=dv=64) -> scores with
half-causal mask (rows <1024 attend cols <1024 dense; rows >=1024 causal)
-> softmax -> out proj.

Sharding: 8 cores = 2 batches x 4 head-groups (4 heads each).  Each core
computes its batch's full QKV for its 4 heads (column-sharded W), attention
for those heads, and a partial output projection (row-sharded Wo).  Host
sums the 4 partials per batch.

v3 changes vs v2 (152.3us -> 122.4us):
 - scores in fp8 e4m3 via DoubleRow perf mode with a stride-0 k-tile slot:
   both DR slots read the same data, computing 2x(k.q) at 0.5 cyc/row; the
   doubling folds into the exp scale (0.0625).  Halves scores PE time.
 - QKV projections as 3-term hi/lo fp8 DoubleRow (xh@Wh + xl@Wh + xh@Wl,
   weights x16 against e4m3 subnormals, 1/16 folded into the psum copy):
   0.75x the bf16 matmul rows at ~bf16 accuracy (rel err 1.5e-2 vs 2e-2
   gate, measured on HW).
 - PV flipped: out[q,v] = probs[k,q].T @ v[k,v] per 128-q chunk, cost 65
   rows/chunk instead of 128 (the 65-wide v+denominator free dim is the
   cheap side).  Denominator becomes a per-partition scalar: recip +
   tensor_scalar multiply, killing v2's dscr broadcast-DMA machinery.
 - att transposed back via PE transpose (identity moving operand) with the
   bf16 output bitcast into the just-consumed pv psum bank (no extra bank).
 - no y2 partial: the causal staircase finishes head (1,1) q-chunk j at
   kc 8+j, so the full out-projection (both head pairs) rides the diagonal.
 - work-queue scheduler paced to the exp cadence: filler (QKV chunks, out-
   projections) pops between scores/exp emissions under a carry-over row
   budget; PV/norm units release one period after their probs so the
   in-order PE stream never stalls on exp; deadline `ensure` pulls keep
   correctness independent of pacing; half-0 out-projections are rationed
   across the half-1 dense blocks to feed PE through the ACT-bound stretch.

Engine budget (TimelineSim): ACT 98.7us (exp-bound), PE 88.0us,
DVE 55us, Pool 29us -> 122.4us total (drain copies alternate ACT/DVE;
the first two kc's lo-half scores fire in the ramp; blocks 6-7 pop the
work queue unthrottled).
"""

import copy as _copy
import heapq
import sys
from collections import deque

if "/opt/trn_rl_repo" not in sys.path:
    sys.path.insert(0, "/opt/trn_rl_repo")

import ml_dtypes
import numpy as np

import concourse.bass as bass  # noqa: F401 (import registers engines)
import concourse.mybir as mybir
import concourse.tile as tile
from concourse import bacc
from concourse.bass_utils import run_bass_kernel_spmd

f32 = mybir.dt.float32
bf16 = mybir.dt.bfloat16
fp8 = mybir.dt.float8e4
AF = mybir.ActivationFunctionType
OP = mybir.AluOpType
PM = mybir.MatmulPerfMode

D = 1024  # d_model
N = 2048  # n_ctx
HG = 256  # head-group width per core (4 heads x 64)

HEADS = [(0, 0), (0, 1), (1, 0), (1, 1)]  # (hp, par)


def stride0(ap):
    """Turn the first singleton non-partition dim into a stride-0 2-count
    dim (the DoubleRow k-tile slot reading the same data twice)."""
    ap2 = _copy.copy(ap)
    lst = ap2.ap
    for i in range(1, len(lst)):
        if lst[i][1] == 1:
            lst[i] = [0, 2]
            return ap2
    raise RuntimeError(f"no singleton dim in {lst}")


def build_nc():
    nc = bacc.Bacc("TRN2", target_bir_lowering=False, debug=False)

    # x and the QKV weights ship as fp8 hi/lo pairs (weights pre-scaled x16
    # on the host; the 1/16 folds into the psum->sbuf copy)
    xth = nc.declare_dram_parameter("xth", [D, N], fp8, isOutput=False)
    xtl = nc.declare_dram_parameter("xtl", [D, N], fp8, isOutput=False)
    wq8 = nc.declare_dram_parameter("wq8", [D, 2, HG], fp8, isOutput=False)
    wk8 = nc.declare_dram_parameter("wk8", [D, 2, HG], fp8, isOutput=False)
    wv8 = nc.declare_dram_parameter("wv8", [D, 2, HG], fp8, isOutput=False)
    bqd = nc.declare_dram_parameter("bq", [HG], f32, isOutput=False)
    bvd = nc.declare_dram_parameter("bv", [HG], bf16, isOutput=False)
    wo = nc.declare_dram_parameter("wo", [HG, D], bf16, isOutput=False)
    trid = nc.declare_dram_parameter("tri", [128, 128], bf16, isOutput=False)
    idmd = nc.declare_dram_parameter("idm", [128, 128], bf16, isOutput=False)
    onesd = nc.declare_dram_parameter("ones", [512], bf16, isOutput=False)
    y = nc.declare_dram_parameter("y", [N, D], bf16, isOutput=True)

    xth_r = xth[:].rearrange("(c p) n -> p c n", p=128)
    xtl_r = xtl[:].rearrange("(c p) n -> p c n", p=128)
    wq_r = wq8[:].rearrange("(c p) hl m -> p c hl m", p=128)
    wk_r = wk8[:].rearrange("(c p) hl m -> p c hl m", p=128)
    wv_r = wv8[:].rearrange("(c p) hl m -> p c hl m", p=128)
    wo_r = wo[:].rearrange("(c p) n -> p c n", p=128)

    with tile.TileContext(nc) as tc:
        with (
            tc.tile_pool(name="persist", bufs=1) as P1,
            tc.tile_pool(name="xtp", bufs=8) as XTP,
            tc.tile_pool(name="ppool", bufs=40) as PP,
            tc.tile_pool(name="aqp", bufs=4) as AQP,
            tc.tile_pool(name="spp", bufs=2) as SPP,
            tc.tile_pool(name="rp", bufs=4) as RP,
            tc.tile_pool(name="yp", bufs=4) as YP,
            tc.tile_pool(name="ps_s", bufs=2, space="PSUM") as PSS,
            tc.tile_pool(name="ps_pv", bufs=2, space="PSUM") as PSPV,
            tc.tile_pool(name="ps_b", bufs=2, space="PSUM") as PSB,
        ):
            # ---------- persistent tiles ----------
            # [part, c-chunk, hi/lo, cols]: the (c, c+1) pairing for DR slots
            # lives in the c dim; hi/lo selects the split
            wq_sb = P1.tile([128, 8, 2, HG], fp8, tag="wq")
            wk_sb = P1.tile([128, 8, 2, HG], fp8, tag="wk")
            wv_sb = P1.tile([128, 8, 2, HG], fp8, tag="wv")
            wo_sb = P1.tile([128, 2, D], bf16, tag="wo")
            bq_sb = P1.tile([128, 2], f32, tag="bq")
            bv_sb = P1.tile([1, HG], bf16, tag="bv")
            ones_sb = P1.tile([1, 512], bf16, tag="ones")
            tri_sb = P1.tile([128, 128], bf16, tag="tri")
            idm_sb = P1.tile([128, 128], bf16, tag="idm")

            ebase = P1.tile([128, 1024], f32, tag="ebase")
            qT8 = P1.tile([128, 2, N], fp8, tag="qT8")
            kT8 = P1.tile([128, 2, N], fp8, tag="kT8")
            v1 = P1.tile([128, 16, 4, 65], bf16, tag="v1")
            att = P1.tile([128, 2, N], bf16, tag="att")

            # ---------- QKV emitters ----------
            xt_tiles = {}

            def emit_load(n4, split=False):
                ns = slice(512 * n4, 512 * n4 + 512)
                a = XTP.tile([128, 8, 512], fp8, tag="xt", name=f"xth{n4}")
                b_ = XTP.tile([128, 8, 512], fp8, tag="xt", name=f"xtl{n4}")
                for h in range(2):
                    cs = slice(4 * h, 4 * h + 4)
                    nc.sync.dma_start(a[:, cs, :], xth_r[:, cs, ns])
                eng = nc.gpsimd if split else nc.sync
                for h in range(2):
                    cs = slice(4 * h, 4 * h + 4)
                    eng.dma_start(b_[:, cs, :], xtl_r[:, cs, ns])
                xt_tiles[n4] = (a, b_)

            def emit_qk_part(which, n4, m, toff, twid, act_copy=False):
                """Q/K projection for a token sub-range via 3-term hi/lo fp8
                DoubleRow (xh@Wh + xl@Wh + xh@Wl); weights are x16 so the
                psum->sbuf copy scales by 1/16 (and adds bq for Q)."""
                ns = slice(512 * n4 + toff, 512 * n4 + toff + twid)
                xh, xl = xt_tiles[n4]
                msl = slice(128 * m, 128 * m + 128)
                w_sb = wq_sb if which == "q" else wk_sb
                ps = PSB.tile(
                    [128, 512], f32, tag="b", name=f"{which}ps{n4}{m}{toff}"
                )
                terms = [(xh, 0), (xl, 0), (xh, 1)]
                for ti, (xs, hl) in enumerate(terms):
                    for j in range(4):
                        nc.tensor.matmul(
                            ps[:, 0:twid],
                            w_sb[:, 2 * j : 2 * j + 2, hl, msl],
                            xs[:, 2 * j : 2 * j + 2, toff : toff + twid],
                            start=(ti == 0 and j == 0),
                            stop=(ti == 2 and j == 3),
                            perf_mode=PM.DoubleRow,
                        )
                if which == "q":
                    with nc.allow_low_precision(reason="fp8 qT"):
                        if act_copy:
                            nc.scalar.activation(
                                qT8[:, m, ns], ps[:, 0:twid], AF.Identity,
                                bias=bq_sb[:, m : m + 1], scale=1.0 / 16.0,
                            )
                        else:
                            nc.vector.tensor_scalar(
                                out=qT8[:, m, ns], in0=ps[:, 0:twid],
                                scalar1=1.0 / 16.0,
                                scalar2=bq_sb[:, m : m + 1],
                                op0=OP.mult, op1=OP.add,
                            )
                else:
                    with nc.allow_low_precision(reason="fp8 kT"):
                        if act_copy:
                            nc.scalar.mul(kT8[:, m, ns], ps[:, 0:twid], 1.0 / 16.0)
                        else:
                            nc.vector.tensor_scalar(
                                out=kT8[:, m, ns], in0=ps[:, 0:twid],
                                scalar1=1.0 / 16.0, scalar2=None, op0=OP.mult,
                            )

            def emit_q(n4, m, act_copy=False):
                emit_qk_part("q", n4, m, 0, 512, act_copy=act_copy)

            def emit_k(n4, m, act_copy=False):
                emit_qk_part("k", n4, m, 0, 512, act_copy=act_copy)

            def emit_v(s, act_copy=False):
                n4 = s // 4
                xh, xl = xt_tiles[n4]
                so = 128 * (s - 4 * n4)
                ps = PSB.tile([128, 256], f32, tag="b", name=f"vps{s}")
                terms = [(xh, 0), (xl, 0), (xh, 1)]
                for ti, (xs, hl) in enumerate(terms):
                    for j in range(4):
                        nc.tensor.matmul(
                            ps[:],
                            xs[:, 2 * j : 2 * j + 2, so : so + 128],
                            wv_sb[:, 2 * j : 2 * j + 2, hl, :],
                            start=(ti == 0 and j == 0),
                            stop=False,
                            perf_mode=PM.DoubleRow,
                        )
                # bias (x16 on host, like the weights)
                nc.tensor.matmul(
                    ps[:], ones_sb[:, :128], bv_sb[:], start=False, stop=True
                )
                with nc.allow_low_precision(reason="bf16 v1"):
                    src = ps[:].rearrange("p (h d) -> p h d", h=4)
                    if act_copy:
                        nc.scalar.mul(v1[:, s, :, 0:64], src, 1.0 / 16.0)
                    else:
                        nc.vector.tensor_scalar(
                            out=v1[:, s, :, 0:64], in0=src,
                            scalar1=1.0 / 16.0, scalar2=None, op0=OP.mult,
                        )

            # ---------- attention ----------
            probs = {}  # (bi, kc) -> p_t

            sc_tiles = {}

            def emit_scores_piece(bi, hp, par, half, kc, lo, hi):
                """One <=512-wide scores matmul + its exp piece."""
                q0 = 1024 * half
                base = 64 * par
                ensure(("q", 2 * half + lo // 512, hp, 0))
                ensure(("q", 2 * half + lo // 512, hp, 256))
                if hi - lo > 512 - 256:  # piece spans into the next q-256
                    ensure(("q", 2 * half + (hi - 1) // 512, hp, 0))
                    ensure(("q", 2 * half + (hi - 1) // 512, hp, 256))
                ensure(("k", kc // 4, hp, 256 * ((kc % 4) // 2)))
                s_t, p_t = sc_tiles[(bi, kc)]
                lhsT = stride0(kT8[base : base + 64, hp : hp + 1,
                                   128 * kc : 128 * kc + 128])
                rhs = stride0(qT8[base : base + 64, hp : hp + 1,
                                  q0 + lo : q0 + hi])
                nc.tensor.matmul(
                    s_t[:, lo:hi], lhsT, rhs, start=True, stop=True,
                    perf_mode=PM.DoubleRow,
                )
                with nc.allow_low_precision(reason="bf16 probs"):
                    # DR stride-0 doubles the score; 1/16 = 0.5 * 1/8
                    nc.scalar.activation(
                        p_t[:, lo:hi], s_t[:, lo:hi], AF.Exp, scale=0.0625
                    )

            def emit_scores(bi, hp, par, half, kc, split=False):
                q0 = 1024 * half
                base = 64 * par
                o = 0 if (half == 0 or kc < 8) else 128 * (kc - 8)
                s_t = PSS.tile([128, 1024], f32, tag="s", name=f"s{bi}_{kc}")
                p_t = PP.tile([128, 1024], bf16, tag="p", name=f"p{bi}_{kc}")
                sc_tiles[(bi, kc)] = (s_t, p_t)
                probs[(bi, kc)] = p_t
                if split:
                    # B0 ramp compression: emit only the lo half now; the hi
                    # half (needing q(1,0)) is emitted via emit_scores_piece
                    emit_scores_piece(bi, hp, par, half, kc, 0, 512)
                    return
                # correctness guards: prerequisite projections must be emitted
                for n4 in (2 * half, 2 * half + 1):
                    for toff in (0, 256):
                        ensure(("q", n4, hp, toff))
                ensure(("k", kc // 4, hp, 256 * ((kc % 4) // 2)))
                lhsT = stride0(kT8[base : base + 64, hp : hp + 1,
                                   128 * kc : 128 * kc + 128])
                # matmul out must stay within one PSUM bank (<=512 f32)
                mm = [(o, 512), (512, 1024)] if o < 512 else [(o, 1024)]
                for lo, hi in mm:
                    rhs = stride0(qT8[base : base + 64, hp : hp + 1,
                                      q0 + lo : q0 + hi])
                    nc.tensor.matmul(
                        s_t[:, lo:hi], lhsT, rhs, start=True, stop=True,
                        perf_mode=PM.DoubleRow,
                    )
                w_ = 1024 - o
                if half == 1 and kc <= -1:
                    # offload early dense exps (their probs are consumed ~8
                    # periods later, hiding the DVE->Pool chain latency):
                    # DVE stages scores to SBUF, Pool computes base^s
                    s_sb = SPP.tile(
                        [128, 1024], f32, tag="ss", name=f"ss{bi}_{kc}"
                    )
                    nc.vector.tensor_copy(out=s_sb[:, 0:w_], in_=s_t[:, o:1024])
                    with nc.allow_low_precision(reason="bf16 probs"):
                        nc.gpsimd.tensor_tensor(
                            p_t[:, o:1024], ebase[:, 0:w_], s_sb[:, 0:w_],
                            OP.pow,
                        )
                else:
                    with nc.allow_low_precision(reason="bf16 probs"):
                        # DR stride-0 doubles the score; 1/16 = 0.5 * 1/8
                        nc.scalar.activation(
                            p_t[:, o:1024], s_t[:, o:1024], AF.Exp, scale=0.0625
                        )
                if half == 1 and kc >= 8:
                    # causal mask inside the diagonal 128-block. Pool for the
                    # early blocks; DVE for the last two, whose staircase
                    # would otherwise queue behind slow SWDGE y-DMAs on Pool
                    eng = nc.gpsimd
                    eng.tensor_tensor(
                        p_t[:, o : o + 128], p_t[:, o : o + 128], tri_sb[:],
                        OP.mult,
                    )
                probs[(bi, kc)] = p_t

            # per-block deferred transpose state: (pv, attq, qc)
            pend_fin = {}

            def emit_pv(bi, hp, par, half, qc, nkc):
                for kc in range(nkc):
                    ensure(("v", kc))
                h2 = 2 * hp + par
                pv = PSPV.tile([128, 512], f32, tag="pv", name=f"pv{bi}_{qc}")
                col = slice(128 * qc, 128 * qc + 128)
                for kc in range(nkc):
                    nc.tensor.matmul(
                        pv[:, 0:65],
                        probs[(bi, kc)][:, col],
                        v1[:, kc, h2, :],
                        start=(kc == 0),
                        stop=(kc == nkc - 1),
                    )
                return pv

            def emit_norm(bi, qc, pv, act=False):
                r = RP.tile([128, 1], f32, tag="r", name=f"r{bi}_{qc}")
                nc.vector.reciprocal(r[:], pv[:, 64:65])
                attq = AQP.tile([128, 64], bf16, tag="aq", name=f"aq{bi}_{qc}")
                with nc.allow_low_precision(reason="bf16 attq"):
                    if act:
                        # ACT is idle in the tail: out = in * r (per-partition)
                        nc.scalar.activation(
                            attq[:], pv[:, 0:64], AF.Copy, scale=r[:]
                        )
                    else:
                        nc.vector.tensor_scalar(
                            out=attq[:], in0=pv[:, 0:64], scalar1=r[:],
                            scalar2=None, op0=OP.mult,
                        )
                return attq

            def emit_finish(hp, par, half, qc, pv, attq, act=False):
                # transpose attq -> [64,128] into the dead pv bank, then copy
                base = 64 * par
                q0 = 1024 * half
                trout = pv[base : base + 64, 128:192].bitcast(bf16)
                nc.tensor.matmul(
                    trout, attq[:], idm_sb[:], start=True, stop=True,
                    is_transpose=True,
                )
                with nc.allow_low_precision(reason="bf16 att"):
                    dst = att[base : base + 64, hp,
                              q0 + 128 * qc : q0 + 128 * qc + 128]
                    if act:
                        nc.scalar.copy(out=dst, in_=trout)
                    else:
                        nc.vector.tensor_copy(out=dst, in_=trout)

            # ---------- output projection ----------
            yt_tiles = {}

            def emit_outproj_seg(s, nseg, late=False):
                if nseg == 0:
                    yt_tiles[s] = YP.tile([128, D], bf16, tag="y", name=f"yt{s}")
                yt = yt_tiles[s]
                ps = PSB.tile([128, 512], f32, tag="b", name=f"yps{s}_{nseg}")
                for hp in range(2):
                    nc.tensor.matmul(
                        ps[:],
                        att[:, hp, 128 * s : 128 * s + 128],
                        wo_sb[:, hp, 512 * nseg : 512 * nseg + 512],
                        start=(hp == 0),
                        stop=(hp == 1),
                    )
                sl = slice(512 * nseg, 512 * nseg + 512)
                with nc.allow_low_precision(reason="bf16 y"):
                    if s >= 12 and (s + nseg) % 2 == 0:
                        nc.scalar.copy(out=yt[:, sl], in_=ps[:])
                    else:
                        nc.vector.tensor_copy(out=yt[:, sl], in_=ps[:])
                if nseg == 1:
                    del yt_tiles[s]
                    # dual-queue issue so the sequencer cost doesn't serialize
                    # the drain; keep the slower SWDGE path off the last chunks
                    eng = nc.gpsimd if not late else nc.sync
                    eng.dma_start(y[128 * s : 128 * s + 128, :], yt[:])

            # ============================================================
            # work queue
            # ============================================================
            work = deque()  # (rows, fn, key) — filler units
            done = set()  # keys of emitted filler units
            pv_q = []  # heap of (release_period, seq, rows, fn)
            pv_seq = [0]
            period = [0]
            acct = [0.0]  # carry-over PE-row budget

            def run_unit(rows, fn, key):
                if key is not None:
                    if key in done:
                        return 0
                    done.add(key)
                fn()
                return rows

            def ensure(key):
                """Emit a queued filler unit NOW if it hasn't run yet —
                correctness guard so pacing can never reorder a consumer
                ahead of its producer."""
                if key in done:
                    return
                for i, (rows, fn, k) in enumerate(work):
                    if k == key:
                        del work[i]
                        run_unit(rows, fn, key)
                        acct[0] -= rows
                        return
                raise RuntimeError(f"missing unit {key}")

            def pv_push(release, rows_fn):
                rows, fn = rows_fn
                heapq.heappush(pv_q, (release, pv_seq[0], rows, fn))
                pv_seq[0] += 1

            def pop_rows(alloc):
                # released PV/op units first (their release period guarantees
                # deps are done, so they never stall the in-order PE stream),
                # then ready filler while the carry-over account affords it
                acct[0] = min(acct[0] + alloc, 1.5 * alloc if alloc > 0 else acct[0])
                while pv_q and pv_q[0][0] <= period[0]:
                    _, _, rows, fn = heapq.heappop(pv_q)
                    fn()
                    acct[0] -= rows
                while work and work[0][0] <= acct[0]:
                    rows, fn, key = work.popleft()
                    acct[0] -= run_unit(rows, fn, key)

            def pv_unit(bi, hp, par, half, qc, nkc):
                act = bi >= 7 and qc % 2 == 0

                def fn():
                    pv = emit_pv(bi, hp, par, half, qc, nkc)
                    attq = emit_norm(bi, qc, pv, act=act)
                    if bi in pend_fin:
                        ppv, pattq, pqc = pend_fin.pop(bi)
                        emit_finish(hp, par, half, pqc, ppv, pattq, act=act)
                        after_finish(bi, half, pqc)
                    pend_fin[bi] = (pv, attq, qc)

                return (65 * nkc + 192, fn)

            def flush_unit(bi, hp, par, half):
                act = False

                def fn():
                    if bi in pend_fin:
                        ppv, pattq, pqc = pend_fin.pop(bi)
                        emit_finish(hp, par, half, pqc, ppv, pattq, act=act)
                        after_finish(bi, half, pqc)

                return (192, fn)

            def after_finish(bi, half, qc):
                # out-projections unlock when the LAST head (block bi=3 for
                # half 0, bi=7 for half 1) lands its att chunk
                if bi == 3 and qc == 7:
                    # ration the deadline-free half-0 out-projections across
                    # blocks 4-6 so the diag stretches keep PE fed
                    for i in range(16):
                        s, nseg = i // 2, i % 2
                        pv_push(
                            period[0] + 2 + 3 * i,
                            (1024, lambda s=s, n=nseg: emit_outproj_seg(s, n)),
                        )
                elif bi == 7:
                    s = 8 + qc
                    for nseg in range(2):
                        pv_push(
                            period[0],
                            (1024, lambda s=s, n=nseg:
                             emit_outproj_seg(s, n, late=(s >= 12))),
                        )

            # ============================================================
            # emission schedule
            # ============================================================
            # --- ramp: stream x/weights, QKV chunks 0-1 ---
            nc.gpsimd.memzero(ones_sb[:])
            with nc.allow_low_precision(reason="bf16 ones"):
                nc.gpsimd.tensor_scalar_add(ones_sb[:], ones_sb[:], 1.0)
            nc.gpsimd.memset(ebase[:], float(np.exp(0.0625)))
            nc.scalar.dma_start(wq_sb[:, 0:4, :, :], wq_r[:, 0:4, :, :])
            emit_load(0, split=True)
            nc.scalar.dma_start(wq_sb[:, 4:8, :, :], wq_r[:, 4:8, :, :])
            nc.scalar.dma_start(wk_sb[:, 0:4, :, :], wk_r[:, 0:4, :, :])
            nc.scalar.dma_start(wk_sb[:, 4:8, :, :], wk_r[:, 4:8, :, :])
            nc.scalar.dma_start(wv_sb[:], wv_r[:])
            nc.gpsimd.dma_start(bq_sb[:], bqd[:].rearrange("(m p) -> p m", p=128))
            nc.gpsimd.dma_start(bv_sb[:], bvd[None, :])
            nc.scalar.dma_start(tri_sb[:], trid[:])
            nc.scalar.dma_start(idm_sb[:], idmd[:])
            # warmup matmuls: climb the PE p-state while DMAs stream
            for w in range(8):
                junk = PSB.tile([128, 512], f32, tag="b", name=f"warm{w}")
                nc.tensor.matmul(
                    junk[:], ones_sb[0:1, :128], ones_sb[:], start=True, stop=True
                )
            # minimal pre-B0 projections: q(0,0) and k(0,0), then the first
            # two kc's lo-half scores fire before q(1,0) is even projected
            emit_q(0, 0, act_copy=True)
            emit_k(0, 0, act_copy=True)
            emit_load(1)
            done.update({("q", 0, 0, 0), ("q", 0, 0, 256),
                         ("k", 0, 0, 0), ("k", 0, 0, 256)})
            hp0, par0, _ = None, None, None
            emit_scores(0, 0, 0, 0, 0, split=True)
            emit_scores(0, 0, 0, 0, 1, split=True)
            emit_q(1, 0, act_copy=True)
            done.update({("q", 1, 0, 0), ("q", 1, 0, 256)})
            nc.sync.dma_start(v1[:, :, :, 64:65], onesd[0:64].partition_broadcast(128))
            emit_load(2)
            emit_load(3)
            nc.scalar.dma_start(wo_sb[:], wo_r[:])

            def push_qk(which, n4, m):
                for toff in (0, 256):
                    work.append(
                        (1536,
                         lambda n4=n4, m=m, toff=toff:
                         emit_qk_part(which, n4, m, toff, 256),
                         (which, n4, m, toff))
                    )

            # --- remaining QKV queued as filler, in deadline order ---
            # v0-7 + k(1,0): B0; m=1 units: B2 kc0; q n4 2-3: B4; v8-15: B5
            def push_v(s):
                work.append((1792, lambda s=s: emit_v(s), ("v", s)))

            push_v(0)
            push_v(1)
            push_qk("k", 1, 0)
            for s in range(2, 8):
                push_v(s)
            push_qk("q", 0, 1)
            push_qk("k", 0, 1)
            push_qk("q", 1, 1)
            push_qk("k", 1, 1)
            push_qk("q", 2, 0)
            push_qk("q", 3, 0)
            push_qk("q", 2, 1)
            push_qk("q", 3, 1)
            push_qk("k", 2, 0)
            push_qk("k", 3, 0)
            for s in range(8, 12):
                push_v(s)
            push_qk("k", 2, 1)
            push_qk("k", 3, 1)
            for s in range(12, 16):
                push_v(s)

            # --- main blocks ---
            blocks = [(hp, par, half) for half in (0, 1) for hp, par in HEADS]
            for bi, (hp, par, half) in enumerate(blocks):
                nkcs = 8 if half == 0 else 16
                for kc in range(nkcs):
                    o = 0 if (half == 0 or kc < 8) else 128 * (kc - 8)
                    if bi == 0 and kc < 2:
                        # lo halves already emitted in the ramp
                        emit_scores_piece(bi, hp, par, half, kc, 512, 1024)
                    else:
                        emit_scores(bi, hp, par, half, kc)
                    if half == 0:
                        if kc == 7:
                            # spread the 8 units across the next block's
                            # periods to avoid a block-boundary burst
                            for qc in range(8):
                                pv_push(
                                    period[0] + 1 + (3 * qc) // 4,
                                    pv_unit(bi, hp, par, half, qc, 8),
                                )
                            pv_push(period[0] + 7, flush_unit(bi, hp, par, half))
                    else:
                        if kc >= 8:
                            qc = kc - 8
                            pv_push(
                                period[0] + 1,
                                pv_unit(bi, hp, par, half, qc, kc + 1),
                            )
                            if kc == 15:
                                pv_push(
                                    period[0] + 1, flush_unit(bi, hp, par, half)
                                )
                    # pace the queue at ~1.05x the exp cadence so ACT (not PE)
                    # absorbs scheduling jitter; scores rows count against it
                    w_ = 1024 - o
                    exp_rows = (0.833 * w_ + 185.0) / 0.4167
                    alloc = 0.95 * exp_rows - w_ // 2
                    if bi >= 6:
                        alloc = max(alloc, 8000.0)
                    pop_rows(alloc)
                    period[0] += 1

            # --- drain ---
            period[0] += 1000
            while pv_q or work:
                while pv_q:
                    _, _, rows, fn = heapq.heappop(pv_q)
                    fn()
                while work:
                    rows, fn = work.popleft()
                    fn()

    nc.compile()
    return nc


_NC = None


def _get_nc():
    global _NC
    if _NC is None:
        _NC = build_nc()
    return _NC


def make_in_maps(x, Wq, bq, Wk, bk, Wv, bv, Wo):
    _get_nc()
    bf = ml_dtypes.bfloat16
    e4 = ml_dtypes.float8_e4m3fn
    x = np.asarray(x, np.float32)
    kk = np.arange(128)[:, None]
    qp = np.arange(128)[None, :]
    tri = (kk <= qp).astype(bf)
    idm = np.eye(128, dtype=np.float32).astype(bf)
    ones = np.ones(512, bf)

    def hilo(a):
        h = a.astype(e4)
        l = (a - h.astype(np.float32)).astype(e4)
        return h, l

    def w8(W, sl):
        # [D, 2, HG]: hi/lo of 16*W
        h, l = hilo(np.asarray(W, np.float32)[:, sl] * 16.0)
        return np.ascontiguousarray(np.stack([h, l], axis=1))

    in_maps = []
    for core in range(8):
        b, g = core // 4, core % 4
        sl = slice(HG * g, HG * (g + 1))
        xh, xl = hilo(x[b].T)
        in_maps.append(
            {
                "xth": np.ascontiguousarray(xh),
                "xtl": np.ascontiguousarray(xl),
                "wq8": w8(Wq, sl),
                "wk8": w8(Wk, sl),
                "wv8": w8(Wv, sl),
                "bq": np.ascontiguousarray(np.asarray(bq, np.float32)[sl]),
                "bv": np.ascontiguousarray(
                    (np.asarray(bv, np.float32)[sl] * 16.0).astype(bf)
                ),
                "wo": np.ascontiguousarray(np.asarray(Wo, np.float32)[sl, :].astype(bf)),
                "tri": tri,
                "idm": idm,
                "ones": ones,
            }
        )
    return in_maps


def kernel(x, Wq, bq, Wk, bk, Wv, bv, Wo, _trace=False, _trace_kwargs=None):
    nc = _get_nc()
    in_maps = make_in_maps(x, Wq, bq, Wk, bk, Wv, bv, Wo)
    res = run_bass_kernel_spmd(
        nc, in_maps, list(range(8)), trace=_trace, **(_trace_kwargs or {})
    )
    out = np.zeros((2, N, D), np.float64)
    for core in range(8):
        out[core // 4] += np.asarray(res.results[core]["y"], np.float64)
    yf = out.astype(np.float32)
    if _trace:
        return yf, res
    return yf


# revision 7
# speedup vs baseline: 1.0268x; 1.0016x over previous
"""Sparse (half-causal) multi-head attention on 8 Trainium2 NeuronCores, v3.

Problem: x[2,2048,1024] -> QKV proj (16 heads, dk=dv=64) -> scores with
half-causal mask (rows <1024 attend cols <1024 dense; rows >=1024 causal)
-> softmax -> out proj.

Sharding: 8 cores = 2 batches x 4 head-groups (4 heads each).  Each core
computes its batch's full QKV for its 4 heads (column-sharded W), attention
for those heads, and a partial output projection (row-sharded Wo).  Host
sums the 4 partials per batch.

v3 changes vs v2 (152.3us -> 122.1us):
 - scores in fp8 e4m3 via DoubleRow perf mode with a stride-0 k-tile slot:
   both DR slots read the same data, computing 2x(k.q) at 0.5 cyc/row; the
   doubling folds into the exp scale (0.0625).  Halves scores PE time.
 - QKV projections as 3-term hi/lo fp8 DoubleRow (xh@Wh + xl@Wh + xh@Wl,
   weights x16 against e4m3 subnormals, 1/16 folded into the psum copy):
   0.75x the bf16 matmul rows at ~bf16 accuracy (rel err 1.5e-2 vs 2e-2
   gate, measured on HW).
 - PV flipped: out[q,v] = probs[k,q].T @ v[k,v] per 128-q chunk, cost 65
   rows/chunk instead of 128 (the 65-wide v+denominator free dim is the
   cheap side).  Denominator becomes a per-partition scalar: recip +
   tensor_scalar multiply, killing v2's dscr broadcast-DMA machinery.
 - att transposed back via PE transpose (identity moving operand) with the
   bf16 output bitcast into the just-consumed pv psum bank (no extra bank).
 - no y2 partial: the causal staircase finishes head (1,1) q-chunk j at
   kc 8+j, so the full out-projection (both head pairs) rides the diagonal.
 - work-queue scheduler paced to the exp cadence: filler (QKV chunks, out-
   projections) pops between scores/exp emissions under a carry-over row
   budget; PV/norm units release one period after their probs so the
   in-order PE stream never stalls on exp; deadline `ensure` pulls keep
   correctness independent of pacing; half-0 out-projections are rationed
   across the half-1 dense blocks to feed PE through the ACT-bound stretch.

Engine budget (TimelineSim): ACT 98.7us (exp-bound), PE 88.0us,
DVE 55us, Pool 29us -> 122.1us total (drain copies alternate ACT/DVE;
the first two kc's lo-half scores fire in the ramp; blocks 6-7 pop the
work queue unthrottled; the last block's final staircase steps release
same-period to shorten the drain).
"""

import copy as _copy
import heapq
import sys
from collections import deque

if "/opt/trn_rl_repo" not in sys.path:
    sys.path.insert(0, "/opt/trn_rl_repo")

import ml_dtypes
import numpy as np

import concourse.bass as bass  # noqa: F401 (import registers engines)
import concourse.mybir as mybir
import concourse.tile as tile
from concourse import bacc
from concourse.bass_utils import run_bass_kernel_spmd

f32 = mybir.dt.float32
bf16 = mybir.dt.bfloat16
fp8 = mybir.dt.float8e4
AF = mybir.ActivationFunctionType
OP = mybir.AluOpType
PM = mybir.MatmulPerfMode

D = 1024  # d_model
N = 2048  # n_ctx
HG = 256  # head-group width per core (4 heads x 64)

HEADS = [(0, 0), (0, 1), (1, 0), (1, 1)]  # (hp, par)


def stride0(ap):
    """Turn the first singleton non-partition dim into a stride-0 2-count
    dim (the DoubleRow k-tile slot reading the same data twice)."""
    ap2 = _copy.copy(ap)
    lst = ap2.ap
    for i in range(1, len(lst)):
        if lst[i][1] == 1:
            lst[i] = [0, 2]
            return ap2
    raise RuntimeError(f"no singleton dim in {lst}")


def build_nc():
    nc = bacc.Bacc("TRN2", target_bir_lowering=False, debug=False)

    # x and the QKV weights ship as fp8 hi/lo pairs (weights pre-scaled x16
    # on the host; the 1/16 folds into the psum->sbuf copy)
    xth = nc.declare_dram_parameter("xth", [D, N], fp8, isOutput=False)
    xtl = nc.declare_dram_parameter("xtl", [D, N], fp8, isOutput=False)
    wq8 = nc.declare_dram_parameter("wq8", [D, 2, HG], fp8, isOutput=False)
    wk8 = nc.declare_dram_parameter("wk8", [D, 2, HG], fp8, isOutput=False)
    wv8 = nc.declare_dram_parameter("wv8", [D, 2, HG], fp8, isOutput=False)
    bqd = nc.declare_dram_parameter("bq", [HG], f32, isOutput=False)
    bvd = nc.declare_dram_parameter("bv", [HG], bf16, isOutput=False)
    wo = nc.declare_dram_parameter("wo", [HG, D], bf16, isOutput=False)
    trid = nc.declare_dram_parameter("tri", [128, 128], bf16, isOutput=False)
    idmd = nc.declare_dram_parameter("idm", [128, 128], bf16, isOutput=False)
    onesd = nc.declare_dram_parameter("ones", [512], bf16, isOutput=False)
    y = nc.declare_dram_parameter("y", [N, D], bf16, isOutput=True)

    xth_r = xth[:].rearrange("(c p) n -> p c n", p=128)
    xtl_r = xtl[:].rearrange("(c p) n -> p c n", p=128)
    wq_r = wq8[:].rearrange("(c p) hl m -> p c hl m", p=128)
    wk_r = wk8[:].rearrange("(c p) hl m -> p c hl m", p=128)
    wv_r = wv8[:].rearrange("(c p) hl m -> p c hl m", p=128)
    wo_r = wo[:].rearrange("(c p) n -> p c n", p=128)

    with tile.TileContext(nc) as tc:
        with (
            tc.tile_pool(name="persist", bufs=1) as P1,
            tc.tile_pool(name="xtp", bufs=8) as XTP,
            tc.tile_pool(name="ppool", bufs=40) as PP,
            tc.tile_pool(name="aqp", bufs=4) as AQP,
            tc.tile_pool(name="spp", bufs=2) as SPP,
            tc.tile_pool(name="rp", bufs=4) as RP,
            tc.tile_pool(name="yp", bufs=4) as YP,
            tc.tile_pool(name="ps_s", bufs=2, space="PSUM") as PSS,
            tc.tile_pool(name="ps_pv", bufs=2, space="PSUM") as PSPV,
            tc.tile_pool(name="ps_b", bufs=2, space="PSUM") as PSB,
        ):
            # ---------- persistent tiles ----------
            # [part, c-chunk, hi/lo, cols]: the (c, c+1) pairing for DR slots
            # lives in the c dim; hi/lo selects the split
            wq_sb = P1.tile([128, 8, 2, HG], fp8, tag="wq")
            wk_sb = P1.tile([128, 8, 2, HG], fp8, tag="wk")
            wv_sb = P1.tile([128, 8, 2, HG], fp8, tag="wv")
            wo_sb = P1.tile([128, 2, D], bf16, tag="wo")
            bq_sb = P1.tile([128, 2], f32, tag="bq")
            bv_sb = P1.tile([1, HG], bf16, tag="bv")
            ones_sb = P1.tile([1, 512], bf16, tag="ones")
            tri_sb = P1.tile([128, 128], bf16, tag="tri")
            idm_sb = P1.tile([128, 128], bf16, tag="idm")

            ebase = P1.tile([128, 1024], f32, tag="ebase")
            qT8 = P1.tile([128, 2, N], fp8, tag="qT8")
            kT8 = P1.tile([128, 2, N], fp8, tag="kT8")
            v1 = P1.tile([128, 16, 4, 65], bf16, tag="v1")
            att = P1.tile([128, 2, N], bf16, tag="att")

            # ---------- QKV emitters ----------
            xt_tiles = {}

            def emit_load(n4, split=False):
                ns = slice(512 * n4, 512 * n4 + 512)
                a = XTP.tile([128, 8, 512], fp8, tag="xt", name=f"xth{n4}")
                b_ = XTP.tile([128, 8, 512], fp8, tag="xt", name=f"xtl{n4}")
                for h in range(2):
                    cs = slice(4 * h, 4 * h + 4)
                    nc.sync.dma_start(a[:, cs, :], xth_r[:, cs, ns])
                eng = nc.gpsimd if split else nc.sync
                for h in range(2):
                    cs = slice(4 * h, 4 * h + 4)
                    eng.dma_start(b_[:, cs, :], xtl_r[:, cs, ns])
                xt_tiles[n4] = (a, b_)

            def emit_qk_part(which, n4, m, toff, twid, act_copy=False):
                """Q/K projection for a token sub-range via 3-term hi/lo fp8
                DoubleRow (xh@Wh + xl@Wh + xh@Wl); weights are x16 so the
                psum->sbuf copy scales by 1/16 (and adds bq for Q)."""
                ns = slice(512 * n4 + toff, 512 * n4 + toff + twid)
                xh, xl = xt_tiles[n4]
                msl = slice(128 * m, 128 * m + 128)
                w_sb = wq_sb if which == "q" else wk_sb
                ps = PSB.tile(
                    [128, 512], f32, tag="b", name=f"{which}ps{n4}{m}{toff}"
                )
                terms = [(xh, 0), (xl, 0), (xh, 1)]
                for ti, (xs, hl) in enumerate(terms):
                    for j in range(4):
                        nc.tensor.matmul(
                            ps[:, 0:twid],
                            w_sb[:, 2 * j : 2 * j + 2, hl, msl],
                            xs[:, 2 * j : 2 * j + 2, toff : toff + twid],
                            start=(ti == 0 and j == 0),
                            stop=(ti == 2 and j == 3),
                            perf_mode=PM.DoubleRow,
                        )
                if which == "q":
                    with nc.allow_low_precision(reason="fp8 qT"):
                        if act_copy:
                            nc.scalar.activation(
                                qT8[:, m, ns], ps[:, 0:twid], AF.Identity,
                                bias=bq_sb[:, m : m + 1], scale=1.0 / 16.0,
                            )
                        else:
                            nc.vector.tensor_scalar(
                                out=qT8[:, m, ns], in0=ps[:, 0:twid],
                                scalar1=1.0 / 16.0,
                                scalar2=bq_sb[:, m : m + 1],
                                op0=OP.mult, op1=OP.add,
                            )
                else:
                    with nc.allow_low_precision(reason="fp8 kT"):
                        if act_copy:
                            nc.scalar.mul(kT8[:, m, ns], ps[:, 0:twid], 1.0 / 16.0)
                        else:
                            nc.vector.tensor_scalar(
                                out=kT8[:, m, ns], in0=ps[:, 0:twid],
                                scalar1=1.0 / 16.0, scalar2=None, op0=OP.mult,
                            )

            def emit_q(n4, m, act_copy=False):
                emit_qk_part("q", n4, m, 0, 512, act_copy=act_copy)

            def emit_k(n4, m, act_copy=False):
                emit_qk_part("k", n4, m, 0, 512, act_copy=act_copy)

            def emit_v(s, act_copy=False):
                n4 = s // 4
                xh, xl = xt_tiles[n4]
                so = 128 * (s - 4 * n4)
                ps = PSB.tile([128, 256], f32, tag="b", name=f"vps{s}")
                terms = [(xh, 0), (xl, 0), (xh, 1)]
                for ti, (xs, hl) in enumerate(terms):
                    for j in range(4):
                        nc.tensor.matmul(
                            ps[:],
                            xs[:, 2 * j : 2 * j + 2, so : so + 128],
                            wv_sb[:, 2 * j : 2 * j + 2, hl, :],
                            start=(ti == 0 and j == 0),
                            stop=False,
                            perf_mode=PM.DoubleRow,
                        )
                # bias (x16 on host, like the weights)
                nc.tensor.matmul(
                    ps[:], ones_sb[:, :128], bv_sb[:], start=False, stop=True
                )
                with nc.allow_low_precision(reason="bf16 v1"):
                    src = ps[:].rearrange("p (h d) -> p h d", h=4)
                    if act_copy:
                        nc.scalar.mul(v1[:, s, :, 0:64], src, 1.0 / 16.0)
                    else:
                        nc.vector.tensor_scalar(
                            out=v1[:, s, :, 0:64], in0=src,
                            scalar1=1.0 / 16.0, scalar2=None, op0=OP.mult,
                        )

            # ---------- attention ----------
            probs = {}  # (bi, kc) -> p_t

            sc_tiles = {}

            def emit_scores_piece(bi, hp, par, half, kc, lo, hi):
                """One <=512-wide scores matmul + its exp piece."""
                q0 = 1024 * half
                base = 64 * par
                ensure(("q", 2 * half + lo // 512, hp, 0))
                ensure(("q", 2 * half + lo // 512, hp, 256))
                if hi - lo > 512 - 256:  # piece spans into the next q-256
                    ensure(("q", 2 * half + (hi - 1) // 512, hp, 0))
                    ensure(("q", 2 * half + (hi - 1) // 512, hp, 256))
                ensure(("k", kc // 4, hp, 256 * ((kc % 4) // 2)))
                s_t, p_t = sc_tiles[(bi, kc)]
                lhsT = stride0(kT8[base : base + 64, hp : hp + 1,
                                   128 * kc : 128 * kc + 128])
                rhs = stride0(qT8[base : base + 64, hp : hp + 1,
                                  q0 + lo : q0 + hi])
                nc.tensor.matmul(
                    s_t[:, lo:hi], lhsT, rhs, start=True, stop=True,
                    perf_mode=PM.DoubleRow,
                )
                with nc.allow_low_precision(reason="bf16 probs"):
                    # DR stride-0 doubles the score; 1/16 = 0.5 * 1/8
                    nc.scalar.activation(
                        p_t[:, lo:hi], s_t[:, lo:hi], AF.Exp, scale=0.0625
                    )

            def emit_scores(bi, hp, par, half, kc, split=False):
                q0 = 1024 * half
                base = 64 * par
                o = 0 if (half == 0 or kc < 8) else 128 * (kc - 8)
                s_t = PSS.tile([128, 1024], f32, tag="s", name=f"s{bi}_{kc}")
                p_t = PP.tile([128, 1024], bf16, tag="p", name=f"p{bi}_{kc}")
                sc_tiles[(bi, kc)] = (s_t, p_t)
                probs[(bi, kc)] = p_t
                if split:
                    # B0 ramp compression: emit only the lo half now; the hi
                    # half (needing q(1,0)) is emitted via emit_scores_piece
                    emit_scores_piece(bi, hp, par, half, kc, 0, 512)
                    return
                # correctness guards: prerequisite projections must be emitted
                for n4 in (2 * half, 2 * half + 1):
                    for toff in (0, 256):
                        ensure(("q", n4, hp, toff))
                ensure(("k", kc // 4, hp, 256 * ((kc % 4) // 2)))
                lhsT = stride0(kT8[base : base + 64, hp : hp + 1,
                                   128 * kc : 128 * kc + 128])
                # matmul out must stay within one PSUM bank (<=512 f32)
                mm = [(o, 512), (512, 1024)] if o < 512 else [(o, 1024)]
                for lo, hi in mm:
                    rhs = stride0(qT8[base : base + 64, hp : hp + 1,
                                      q0 + lo : q0 + hi])
                    nc.tensor.matmul(
                        s_t[:, lo:hi], lhsT, rhs, start=True, stop=True,
                        perf_mode=PM.DoubleRow,
                    )
                w_ = 1024 - o
                if half == 1 and kc <= -1:
                    # offload early dense exps (their probs are consumed ~8
                    # periods later, hiding the DVE->Pool chain latency):
                    # DVE stages scores to SBUF, Pool computes base^s
                    s_sb = SPP.tile(
                        [128, 1024], f32, tag="ss", name=f"ss{bi}_{kc}"
                    )
                    nc.vector.tensor_copy(out=s_sb[:, 0:w_], in_=s_t[:, o:1024])
                    with nc.allow_low_precision(reason="bf16 probs"):
                        nc.gpsimd.tensor_tensor(
                            p_t[:, o:1024], ebase[:, 0:w_], s_sb[:, 0:w_],
                            OP.pow,
                        )
                else:
                    with nc.allow_low_precision(reason="bf16 probs"):
                        # DR stride-0 doubles the score; 1/16 = 0.5 * 1/8
                        nc.scalar.activation(
                            p_t[:, o:1024], s_t[:, o:1024], AF.Exp, scale=0.0625
                        )
                if half == 1 and kc >= 8:
                    # causal mask inside the diagonal 128-block. Pool for the
                    # early blocks; DVE for the last two, whose staircase
                    # would otherwise queue behind slow SWDGE y-DMAs on Pool
                    eng = nc.gpsimd
                    eng.tensor_tensor(
                        p_t[:, o : o + 128], p_t[:, o : o + 128], tri_sb[:],
                        OP.mult,
                    )
                probs[(bi, kc)] = p_t

            # per-block deferred transpose state: (pv, attq, qc)
            pend_fin = {}

            def emit_pv(bi, hp, par, half, qc, nkc):
                for kc in range(nkc):
                    ensure(("v", kc))
                h2 = 2 * hp + par
                pv = PSPV.tile([128, 512], f32, tag="pv", name=f"pv{bi}_{qc}")
                col = slice(128 * qc, 128 * qc + 128)
                for kc in range(nkc):
                    nc.tensor.matmul(
                        pv[:, 0:65],
                        probs[(bi, kc)][:, col],
                        v1[:, kc, h2, :],
                        start=(kc == 0),
                        stop=(kc == nkc - 1),
                    )
                return pv

            def emit_norm(bi, qc, pv, act=False):
                r = RP.tile([128, 1], f32, tag="r", name=f"r{bi}_{qc}")
                nc.vector.reciprocal(r[:], pv[:, 64:65])
                attq = AQP.tile([128, 64], bf16, tag="aq", name=f"aq{bi}_{qc}")
                with nc.allow_low_precision(reason="bf16 attq"):
                    if act:
                        # ACT is idle in the tail: out = in * r (per-partition)
                        nc.scalar.activation(
                            attq[:], pv[:, 0:64], AF.Copy, scale=r[:]
                        )
                    else:
                        nc.vector.tensor_scalar(
                            out=attq[:], in0=pv[:, 0:64], scalar1=r[:],
                            scalar2=None, op0=OP.mult,
                        )
                return attq

            def emit_finish(hp, par, half, qc, pv, attq, act=False):
                # transpose attq -> [64,128] into the dead pv bank, then copy
                base = 64 * par
                q0 = 1024 * half
                trout = pv[base : base + 64, 128:192].bitcast(bf16)
                nc.tensor.matmul(
                    trout, attq[:], idm_sb[:], start=True, stop=True,
                    is_transpose=True,
                )
                with nc.allow_low_precision(reason="bf16 att"):
                    dst = att[base : base + 64, hp,
                              q0 + 128 * qc : q0 + 128 * qc + 128]
                    if act:
                        nc.scalar.copy(out=dst, in_=trout)
                    else:
                        nc.vector.tensor_copy(out=dst, in_=trout)

            # ---------- output projection ----------
            yt_tiles = {}

            def emit_outproj_seg(s, nseg, late=False):
                if nseg == 0:
                    yt_tiles[s] = YP.tile([128, D], bf16, tag="y", name=f"yt{s}")
                yt = yt_tiles[s]
                ps = PSB.tile([128, 512], f32, tag="b", name=f"yps{s}_{nseg}")
                for hp in range(2):
                    nc.tensor.matmul(
                        ps[:],
                        att[:, hp, 128 * s : 128 * s + 128],
                        wo_sb[:, hp, 512 * nseg : 512 * nseg + 512],
                        start=(hp == 0),
                        stop=(hp == 1),
                    )
                sl = slice(512 * nseg, 512 * nseg + 512)
                with nc.allow_low_precision(reason="bf16 y"):
                    if s >= 12 and (s + nseg) % 2 == 0:
                        nc.scalar.copy(out=yt[:, sl], in_=ps[:])
                    else:
                        nc.vector.tensor_copy(out=yt[:, sl], in_=ps[:])
                if nseg == 1:
                    del yt_tiles[s]
                    # dual-queue issue so the sequencer cost doesn't serialize
                    # the drain; keep the slower SWDGE path off the last chunks
                    eng = nc.gpsimd if not late else nc.sync
                    eng.dma_start(y[128 * s : 128 * s + 128, :], yt[:])

            # ============================================================
            # work queue
            # ============================================================
            work = deque()  # (rows, fn, key) — filler units
            done = set()  # keys of emitted filler units
            pv_q = []  # heap of (release_period, seq, rows, fn)
            pv_seq = [0]
            period = [0]
            acct = [0.0]  # carry-over PE-row budget

            def run_unit(rows, fn, key):
                if key is not None:
                    if key in done:
                        return 0
                    done.add(key)
                fn()
                return rows

            def ensure(key):
                """Emit a queued filler unit NOW if it hasn't run yet —
                correctness guard so pacing can never reorder a consumer
                ahead of its producer."""
                if key in done:
                    return
                for i, (rows, fn, k) in enumerate(work):
                    if k == key:
                        del work[i]
                        run_unit(rows, fn, key)
                        acct[0] -= rows
                        return
                raise RuntimeError(f"missing unit {key}")

            def pv_push(release, rows_fn):
                rows, fn = rows_fn
                heapq.heappush(pv_q, (release, pv_seq[0], rows, fn))
                pv_seq[0] += 1

            def pop_rows(alloc):
                # released PV/op units first (their release period guarantees
                # deps are done, so they never stall the in-order PE stream),
                # then ready filler while the carry-over account affords it
                acct[0] = min(acct[0] + alloc, 1.5 * alloc if alloc > 0 else acct[0])
                while pv_q and pv_q[0][0] <= period[0]:
                    _, _, rows, fn = heapq.heappop(pv_q)
                    fn()
                    acct[0] -= rows
                while work and work[0][0] <= acct[0]:
                    rows, fn, key = work.popleft()
                    acct[0] -= run_unit(rows, fn, key)

            def pv_unit(bi, hp, par, half, qc, nkc):
                act = bi >= 7 and qc % 2 == 0

                def fn():
                    pv = emit_pv(bi, hp, par, half, qc, nkc)
                    attq = emit_norm(bi, qc, pv, act=act)
                    if bi in pend_fin:
                        ppv, pattq, pqc = pend_fin.pop(bi)
                        emit_finish(hp, par, half, pqc, ppv, pattq, act=act)
                        after_finish(bi, half, pqc)
                    pend_fin[bi] = (pv, attq, qc)

                return (65 * nkc + 192, fn)

            def flush_unit(bi, hp, par, half):
                act = False

                def fn():
                    if bi in pend_fin:
                        ppv, pattq, pqc = pend_fin.pop(bi)
                        emit_finish(hp, par, half, pqc, ppv, pattq, act=act)
                        after_finish(bi, half, pqc)

                return (192, fn)

            def after_finish(bi, half, qc):
                # out-projections unlock when the LAST head (block bi=3 for
                # half 0, bi=7 for half 1) lands its att chunk
                if bi == 3 and qc == 7:
                    # ration the deadline-free half-0 out-projections across
                    # blocks 4-6 so the diag stretches keep PE fed
                    for i in range(16):
                        s, nseg = i // 2, i % 2
                        pv_push(
                            period[0] + 2 + 3 * i,
                            (1024, lambda s=s, n=nseg: emit_outproj_seg(s, n)),
                        )
                elif bi == 7:
                    s = 8 + qc
                    for nseg in range(2):
                        pv_push(
                            period[0],
                            (1024, lambda s=s, n=nseg:
                             emit_outproj_seg(s, n, late=(s >= 12))),
                        )

            # ============================================================
            # emission schedule
            # ============================================================
            # --- ramp: stream x/weights, QKV chunks 0-1 ---
            nc.gpsimd.memzero(ones_sb[:])
            with nc.allow_low_precision(reason="bf16 ones"):
                nc.gpsimd.tensor_scalar_add(ones_sb[:], ones_sb[:], 1.0)
            nc.gpsimd.memset(ebase[:], float(np.exp(0.0625)))
            nc.scalar.dma_start(wq_sb[:, 0:4, :, :], wq_r[:, 0:4, :, :])
            emit_load(0, split=True)
            nc.scalar.dma_start(wq_sb[:, 4:8, :, :], wq_r[:, 4:8, :, :])
            nc.scalar.dma_start(wk_sb[:, 0:4, :, :], wk_r[:, 0:4, :, :])
            nc.scalar.dma_start(wk_sb[:, 4:8, :, :], wk_r[:, 4:8, :, :])
            nc.scalar.dma_start(wv_sb[:], wv_r[:])
            nc.gpsimd.dma_start(bq_sb[:], bqd[:].rearrange("(m p) -> p m", p=128))
            nc.gpsimd.dma_start(bv_sb[:], bvd[None, :])
            nc.scalar.dma_start(tri_sb[:], trid[:])
            nc.scalar.dma_start(idm_sb[:], idmd[:])
            # warmup matmuls: climb the PE p-state while DMAs stream
            for w in range(8):
                junk = PSB.tile([128, 512], f32, tag="b", name=f"warm{w}")
                nc.tensor.matmul(
                    junk[:], ones_sb[0:1, :128], ones_sb[:], start=True, stop=True
                )
            # minimal pre-B0 projections: q(0,0) and k(0,0), then the first
            # two kc's lo-half scores fire before q(1,0) is even projected
            emit_q(0, 0, act_copy=True)
            emit_k(0, 0, act_copy=True)
            emit_load(1)
            done.update({("q", 0, 0, 0), ("q", 0, 0, 256),
                         ("k", 0, 0, 0), ("k", 0, 0, 256)})
            hp0, par0, _ = None, None, None
            emit_scores(0, 0, 0, 0, 0, split=True)
            emit_scores(0, 0, 0, 0, 1, split=True)
            emit_q(1, 0, act_copy=True)
            done.update({("q", 1, 0, 0), ("q", 1, 0, 256)})
            nc.sync.dma_start(v1[:, :, :, 64:65], onesd[0:64].partition_broadcast(128))
            emit_load(2)
            emit_load(3)
            nc.scalar.dma_start(wo_sb[:], wo_r[:])

            def push_qk(which, n4, m):
                for toff in (0, 256):
                    work.append(
                        (1536,
                         lambda n4=n4, m=m, toff=toff:
                         emit_qk_part(which, n4, m, toff, 256),
                         (which, n4, m, toff))
                    )

            # --- remaining QKV queued as filler, in deadline order ---
            # v0-7 + k(1,0): B0; m=1 units: B2 kc0; q n4 2-3: B4; v8-15: B5
            def push_v(s):
                work.append((1792, lambda s=s: emit_v(s), ("v", s)))

            push_v(0)
            push_v(1)
            push_qk("k", 1, 0)
            for s in range(2, 8):
                push_v(s)
            push_qk("q", 0, 1)
            push_qk("k", 0, 1)
            push_qk("q", 1, 1)
            push_qk("k", 1, 1)
            push_qk("q", 2, 0)
            push_qk("q", 3, 0)
            push_qk("q", 2, 1)
            push_qk("q", 3, 1)
            push_qk("k", 2, 0)
            push_qk("k", 3, 0)
            for s in range(8, 12):
                push_v(s)
            push_qk("k", 2, 1)
            push_qk("k", 3, 1)
            for s in range(12, 16):
                push_v(s)

            # --- main blocks ---
            blocks = [(hp, par, half) for half in (0, 1) for hp, par in HEADS]
            for bi, (hp, par, half) in enumerate(blocks):
                nkcs = 8 if half == 0 else 16
                for kc in range(nkcs):
                    o = 0 if (half == 0 or kc < 8) else 128 * (kc - 8)
                    if bi == 0 and kc < 2:
                        # lo halves already emitted in the ramp
                        emit_scores_piece(bi, hp, par, half, kc, 512, 1024)
                    else:
                        emit_scores(bi, hp, par, half, kc)
                    if half == 0:
                        if kc == 7:
                            # spread the 8 units across the next block's
                            # periods to avoid a block-boundary burst
                            for qc in range(8):
                                pv_push(
                                    period[0] + 1 + (3 * qc) // 4,
                                    pv_unit(bi, hp, par, half, qc, 8),
                                )
                            pv_push(period[0] + 7, flush_unit(bi, hp, par, half))
                    else:
                        if kc >= 8:
                            qc = kc - 8
                            # last block's late staircase steps release in
                            # the same period: PE idles there anyway, and
                            # starting the chain earlier shortens the drain
                            lag = 0 if (bi == 7 and kc >= 13) else 1
                            pv_push(
                                period[0] + lag,
                                pv_unit(bi, hp, par, half, qc, kc + 1),
                            )
                            if kc == 15:
                                pv_push(
                                    period[0] + lag,
                                    flush_unit(bi, hp, par, half),
                                )
                    # pace the queue at ~1.05x the exp cadence so ACT (not PE)
                    # absorbs scheduling jitter; scores rows count against it
                    w_ = 1024 - o
                    exp_rows = (0.833 * w_ + 185.0) / 0.4167
                    alloc = 0.95 * exp_rows - w_ // 2
                    if bi >= 6:
                        alloc = max(alloc, 8000.0)
                    pop_rows(alloc)
                    period[0] += 1

            # --- drain ---
            period[0] += 1000
            while pv_q or work:
                while pv_q:
                    _, _, rows, fn = heapq.heappop(pv_q)
                    fn()
                while work:
                    rows, fn = work.popleft()
                    fn()

    nc.compile()
    return nc


_NC = None


def _get_nc():
    global _NC
    if _NC is None:
        _NC = build_nc()
    return _NC


def make_in_maps(x, Wq, bq, Wk, bk, Wv, bv, Wo):
    _get_nc()
    bf = ml_dtypes.bfloat16
    e4 = ml_dtypes.float8_e4m3fn
    x = np.asarray(x, np.float32)
    kk = np.arange(128)[:, None]
    qp = np.arange(128)[None, :]
    tri = (kk <= qp).astype(bf)
    idm = np.eye(128, dtype=np.float32).astype(bf)
    ones = np.ones(512, bf)

    def hilo(a):
        h = a.astype(e4)
        l = (a - h.astype(np.float32)).astype(e4)
        return h, l

    def w8(W, sl):
        # [D, 2, HG]: hi/lo of 16*W
        h, l = hilo(np.asarray(W, np.float32)[:, sl] * 16.0)
        return np.ascontiguousarray(np.stack([h, l], axis=1))

    in_maps = []
    for core in range(8):
        b, g = core // 4, core % 4
        sl = slice(HG * g, HG * (g + 1))
        xh, xl = hilo(x[b].T)
        in_maps.append(
            {
                "xth": np.ascontiguousarray(xh),
                "xtl": np.ascontiguousarray(xl),
                "wq8": w8(Wq, sl),
                "wk8": w8(Wk, sl),
                "wv8": w8(Wv, sl),
                "bq": np.ascontiguousarray(np.asarray(bq, np.float32)[sl]),
                "bv": np.ascontiguousarray(
                    (np.asarray(bv, np.float32)[sl] * 16.0).astype(bf)
                ),
                "wo": np.ascontiguousarray(np.asarray(Wo, np.float32)[sl, :].astype(bf)),
                "tri": tri,
                "idm": idm,
                "ones": ones,
            }
        )
    return in_maps


def kernel(x, Wq, bq, Wk, bk, Wv, bv, Wo, _trace=False, _trace_kwargs=None):
    nc = _get_nc()
    in_maps = make_in_maps(x, Wq, bq, Wk, bk, Wv, bv, Wo)
    res = run_bass_kernel_spmd(
        nc, in_maps, list(range(8)), trace=_trace, **(_trace_kwargs or {})
    )
    out = np.zeros((2, N, D), np.float64)
    for core in range(8):
        out[core // 4] += np.asarray(res.results[core]["y"], np.float64)
    yf = out.astype(np.float32)
    if _trace:
        return yf, res
    return yf
